# revision 37
# baseline (speedup 1.0000x reference)
"""Trainium2 Bass kernel for nn_FFTMemAutoEncoderBranch (retrieval_knn).

Data-parallel over batch: 8 cores x 16 images, no cross-core communication.

Numerics: the problem's top-5 retrieval runs on near-identical queries (white
-noise FFT magnitudes), with 5th/6th similarity gaps down to 7e-6 -- so conv
weights and DFT matrices must act at ~fp32 fidelity while activations tolerate
bf16. Scheme (validated against the reference on HW, 0/128 top-5 flips):
  - activations/staging in bf16
  - every stationary operand (DFT matrices G, conv weights) is split
    W = hi + lo into two bf16 matmuls accumulating in fp32 PSUM
  - retrieval + decoder in fp32

Performance structure (v2): the v1 kernel ran image-major with serial staging,
leaving the PE idle ~80us/image (trace: 1.25ms of gaps in a 3.2ms span, HAM
re-throttling the PE to 1.2GHz for ~85% of the run) and moving ~100MB/core of
SBUF<->SBUF staging in 256-512B DMA descriptors. v2:
  - software-pipelines images with a one-iteration skew: PE order is
    [FFTs1(i+1) | conv1(i) | FFTs2(i+1) | conv2(i) | conv3(i)], so every
    staging chain (mag->DRAM bounce->strip1; pool->align->fold->strip fills)
    runs in the shadow of ~30-90us of matmuls from the neighboring image.
  - staging tiles use pitch-matched padded rows (130-wide for conv2 strips,
    66-wide for conv3) so each strip fill is a handful of multi-KB-contiguous
    descriptors instead of thousands of 256B ones.
  - conv1 folds bias into a 19th K-row (rhs row of ones) and fuses
    relu+x-pool into one DVE scalar_tensor_tensor reading PSUM directly.
  - pool/align/fold/fill chains run per half-image so conv(i) chunk k's rhs
    is staged while chunks k-1 of the same image still run.

Per image: FFT2 as DFT matmuls (z = G x G^T, G = roll(F,128,0)/16, fftshift+
ortho folded in; batch roll done on host), conv1 strips via a DRAM bounce of
the padded 258x258 magnitude image, conv2/conv3 strips via parity-split SBUF
copies, maxpools on DVE, conv3 relu accumulating into q via ACT accum_out.
Retrieval: fp32 sim, top-5 threshold via 5x(reduce_max+mask), masked stable
softmax, mem = values^T @ e^T; decoder collapsed to 3 dense matmuls.
"""

import os
import sys
import numpy as np

for _p in ("/opt/trn_rl_repo", "/root/.axon_site/_ro/trn_rl_repo"):
    if os.path.isdir(_p) and _p not in sys.path:
        sys.path.append(_p)

import concourse.bass as bass
import concourse.mybir as mybir
import concourse.tile as tile
from concourse import bacc
from concourse.bass_utils import run_bass_kernel_spmd

F32 = mybir.dt.float32
BF16 = mybir.dt.bfloat16

N_CORES = 8
B = 128
H = 256

# STAGE: "bf16" (hi/lo-split weights, bf16 activations) | "f32" (all fp32)
STAGE = os.environ.get("K_STAGE", "bf16")
NSPLIT = 2 if STAGE == "bf16" else 1

AluOp = mybir.AluOpType
ActFn = mybir.ActivationFunctionType
AxX = mybir.AxisListType.X


def _sdt():
    return BF16 if STAGE == "bf16" else F32


def _np_sdt():
    if STAGE == "bf16":
        import ml_dtypes
        return ml_dtypes.bfloat16
    return np.float32


# ---------------------------------------------------------------------------
# host-side constant construction
# ---------------------------------------------------------------------------
def _pack2(m):  # [256, N] -> [128, 2, N]
    return np.ascontiguousarray(m.reshape(2, 128, -1).transpose(1, 0, 2))


def _fft_consts():
    k = np.arange(H)
    F = np.exp(-2j * np.pi * np.outer(k, k) / H) / 16.0
    G = np.roll(F, H // 2, axis=0)
    GT = G.T.copy()
    out = {}
    for name, m in (("gre", GT.real), ("gim", GT.imag), ("gimn", -GT.imag)):
        m = m.astype(np.float32)
        if NSPLIT == 1:
            out[name] = _pack2(m)[:, :, None, :]  # [128, 2, 1, 256] f32
        else:
            sdt = _np_sdt()
            hi32 = m.astype(sdt).astype(np.float32)
            lo = (m - hi32).astype(sdt)
            hi = m.astype(sdt)
            # [128, 2, 2, 256]: (part-of-256-rows, half, split, col)
            out[name] = np.stack([_pack2(hi), _pack2(lo)], axis=2)
    return out


T1ORD = (0, 2, 1, 3)  # conv1 M block -> strip row offset t; so that y-pool
# pairs (t0,t1),(t2,t3) become max(partitions 0:64, partitions 64:128)


def _conv1_lhsT(we1, be1):
    # K = (dx, j) packed on partitions 1..18, p = 1 + dx*6 + j (dx-major so
    # each strip1 fill DMA writes a contiguous partition block); partition 0
    # is the bias row (strip1 partition 0 holds ones; engine-op partition
    # bases must be 32-aligned, so the ones memset needs base 0).
    out = np.zeros((19, 1, 128), np.float32)
    for dx in range(3):
        for m in range(128):
            t, co = T1ORD[m // 32], m % 32
            for j in range(6):
                if 0 <= j - t <= 2:
                    out[1 + dx * 6 + j, 0, m] = we1[co, 0, j - t, dx]
    for m in range(128):
        out[0, 0, m] = be1[m % 32]
    return out


def _conv2_lhsT(we2):
    # K layout j-major: k = j*32 + ci (each strip2 fill writes a contiguous
    # partition block)
    out = np.zeros((128, 3, 128), np.float32)
    for dx in range(3):
        for m in range(128):
            t, co = m // 64, m % 64
            for k in range(128):
                ci, j = k % 32, k // 32
                if 0 <= j - t <= 2:
                    out[k, dx, m] = we2[co, ci, j - t, dx]
    return out


def _conv3_lhsT(we3):
    # K layout d-major: k = d*64 + ci
    A = np.zeros((128, 3, 128), np.float32)
    Bm = np.zeros((64, 3, 128), np.float32)
    for dx in range(3):
        for k in range(128):
            ci, d = k % 64, k // 64
            A[k, dx, :] = we3[:, ci, d, dx]
        for ci in range(64):
            Bm[ci, dx, :] = we3[:, ci, 2, dx]
    return A, Bm


def _wsplit(w):
    """[P, D, N] -> [P, D, NSPLIT, N] staging dtype (hi, lo)."""
    sdt = _np_sdt()
    if NSPLIT == 1:
        return w[:, :, None, :].astype(np.float32)
    hi32 = w.astype(sdt).astype(np.float32)
    lo = (w - hi32).astype(sdt)
    return np.stack([w.astype(sdt), lo], axis=2)


def _decoder_mats(wd1, bd1, wd2, bd2, wd3, bd3):
    W1 = np.zeros((128, 256), np.float32)
    for c in range(64):
        for i in range(2):
            for j in range(2):
                W1[:, c * 4 + i * 2 + j] = wd1[:, c, i + 1, j + 1]
    b1 = np.repeat(bd1, 4).astype(np.float32)

    W2 = np.zeros((256, 512), np.float32)
    for c in range(64):
        for ii in range(2):
            for jj in range(2):
                f = c * 4 + ii * 2 + jj
                for c2 in range(32):
                    for y in range(4):
                        ky = y + 1 - 2 * ii
                        if not (0 <= ky <= 3):
                            continue
                        for x in range(4):
                            kx = x + 1 - 2 * jj
                            if 0 <= kx <= 3:
                                W2[f, c2 * 16 + y * 4 + x] = wd2[c, c2, ky, kx]
    b2 = np.repeat(bd2, 16).astype(np.float32)

    W3 = np.zeros((512, 16), np.float32)
    for c2 in range(32):
        for y in range(4):
            for x in range(4):
                g = c2 * 16 + y * 4 + x
                for oy in range(4):
                    ky = y - oy + 1
                    if not (0 <= ky <= 2):
                        continue
                    for ox in range(4):
                        kx = x - ox + 1
                        if 0 <= kx <= 2:
                            W3[g, oy * 4 + ox] = wd3[0, c2, ky, kx]
    b3 = np.full((16,), float(np.asarray(bd3).reshape(-1)[0]), np.float32)
    return W1, b1, W2, b2, W3, b3


def _host_consts(inputs):
    w3a, w3b = _conv3_lhsT(np.asarray(inputs["we3"], np.float32))
    W1, b1, W2, b2, W3, b3 = _decoder_mats(
        np.asarray(inputs["wd1"], np.float32), np.asarray(inputs["bd1"], np.float32),
        np.asarray(inputs["wd2"], np.float32), np.asarray(inputs["bd2"], np.float32),
        np.asarray(inputs["wd3"], np.float32), np.asarray(inputs["bd3"], np.float32))

    keys = np.asarray(inputs["keys"], np.float32)
    values = np.asarray(inputs["values"], np.float32)
    keys_p = np.ones((512, 128), np.float32)
    keys_p[:400] = keys
    values_p = np.zeros((512, 128), np.float32)
    values_p[:400] = values

    c = dict(_fft_consts())
    c.update({
        "w1l": _wsplit(_conv1_lhsT(np.asarray(inputs["we1"], np.float32),
                                   np.asarray(inputs["be1"], np.float32))),
        "w2l": _wsplit(_conv2_lhsT(np.asarray(inputs["we2"], np.float32))),
        "w3a": _wsplit(w3a), "w3b": _wsplit(w3b),
        "cb2": np.tile(np.asarray(inputs["be2"], np.float32), 2).reshape(128, 1),
        "cb3": np.asarray(inputs["be3"], np.float32).reshape(128, 1),
        "keys": np.ascontiguousarray(keys_p.reshape(4, 128, 128).transpose(1, 0, 2)),
        "vals": np.ascontiguousarray(values_p.reshape(4, 128, 128).transpose(1, 0, 2)),
        "ident": np.eye(128, dtype=np.float32),
        "w1d": W1,
        "w2d": np.ascontiguousarray(W2.reshape(2, 128, 4, 128).transpose(1, 0, 2, 3)),
        "w3d": np.ascontiguousarray(W3.reshape(4, 128, 16).transpose(1, 0, 2)),
        "b1d": np.ascontiguousarray(b1.reshape(2, 128).T),
        "b2d": np.ascontiguousarray(b2.reshape(4, 128).T),
        "b3row": b3.reshape(1, 16),
        "ones1": np.ones((1, 16), np.float32),
    })
    return c


def _const_specs():
    s = "stage"
    return {
        "gre": ([128, 2, NSPLIT, 256], s), "gim": ([128, 2, NSPLIT, 256], s),
        "gimn": ([128, 2, NSPLIT, 256], s),
        "w1l": ([19, 1, NSPLIT, 128], s), "w2l": ([128, 3, NSPLIT, 128], s),
        "w3a": ([128, 3, NSPLIT, 128], s), "w3b": ([64, 3, NSPLIT, 128], s),
        "cb2": ([128, 1], "f32"), "cb3": ([128, 1], "f32"),
        "keys": ([128, 4, 128], "f32"), "vals": ([128, 4, 128], "f32"),
        "ident": ([128, 128], "f32"),
        "w1d": ([128, 256], "f32"), "w2d": ([128, 2, 4, 128], "f32"),
        "w3d": ([128, 4, 16], "f32"),
        "b1d": ([128, 2], "f32"), "b2d": ([128, 4], "f32"),
        "b3row": ([1, 16], "f32"), "ones1": ([1, 16], "f32"),
    }


def mk(t, poff, pstep, pcount, fdims, foff=0):
    """Manual AP on tile t (element units; partition pitch from the tile AP)."""
    pitch = t.ap[0][0]
    dims = [[pstep * pitch, pcount]] + [list(d) for d in fdims]
    return bass.AP(t.tensor, t.offset + poff * pitch + foff, dims)


def dramap(t, off, dims):
    return bass.AP(t.tensor, t.offset + off, [list(d) for d in dims])


# ---------------------------------------------------------------------------
# kernel builder
# ---------------------------------------------------------------------------
def build_nc(b_loc=16):
    sdt = _sdt()
    fft_in_dt = BF16 if STAGE == "bf16" else F32
    nc = bacc.Bacc("TRN2", target_bir_lowering=False, debug=False)

    x_in = nc.dram_tensor("x_in", [b_loc, 128, 2, 256], fft_in_dt,
                          kind="ExternalInput")
    out_d = nc.dram_tensor("out", [b_loc, 16], F32, kind="ExternalOutput")
    qdbg_d = (nc.dram_tensor("qdbg", [128, b_loc], F32, kind="ExternalOutput")
              if os.environ.get("K_DBGQ") else None)
    hdbg_d = None
    if os.environ.get("K_DBGH"):
        _ddt = _sdt()
        hdbg_d = {
            "dxm": nc.dram_tensor("dxm", [128, 2, 258], _ddt,
                                  kind="ExternalOutput"),
            "dh1": nc.dram_tensor("dh1", [64, 64, 130], _ddt,
                                  kind="ExternalOutput"),
            "dh2": nc.dram_tensor("dh2", [64, 64, 66], _ddt,
                                  kind="ExternalOutput"),
            "ds2": nc.dram_tensor("ds2", [128, 64, 130], _ddt,
                                  kind="ExternalOutput"),
        }
    const_d = {}
    for name, (shape, kind) in _const_specs().items():
        dt_ = _sdt() if kind == "stage" else F32
        const_d[name] = nc.dram_tensor(name, shape, dt_, kind="ExternalInput")

    with tile.TileContext(nc) as tc:
        from contextlib import ExitStack
        with ExitStack() as ctx:
            cpool = ctx.enter_context(tc.tile_pool(name="consts", bufs=1))
            spool = ctx.enter_context(tc.tile_pool(name="stage", bufs=1))
            xpool = ctx.enter_context(tc.tile_pool(name="xin", bufs=3))
            wpool = ctx.enter_context(tc.tile_pool(name="work", bufs=2))
            rpool = ctx.enter_context(tc.tile_pool(name="ret", bufs=1))
            dpool = ctx.enter_context(tc.tile_pool(name="dram", bufs=1, space="DRAM"))
            fftps = ctx.enter_context(tc.tile_pool(name="fftps", bufs=3, space="PSUM"))
            convps = ctx.enter_context(tc.tile_pool(name="convps", bufs=3, space="PSUM"))
            miscps = ctx.enter_context(tc.tile_pool(name="miscps", bufs=2, space="PSUM"))

            # critical-path consts (FFT G matrices + conv1 weights) load
            # first on the sync queue; everything else goes on the gpsimd
            # queue so image-0's FFT isn't stuck behind ~1MB of decoder
            # weights.
            crit = ("gre", "gim", "gimn", "w1l")
            cs = {}
            for name, (shape, kind) in _const_specs().items():
                dt_ = _sdt() if kind == "stage" else F32
                t = cpool.tile(shape, dt_, name=f"c_{name}")
                if name in crit:
                    nc.sync.dma_start(out=t, in_=const_d[name].ap())
                cs[name] = t

            # fixed stage buffers (all single-buffered; WAR deps order reuse)
            strip1 = spool.tile([19, 2, 32, 256], sdt, name="strip1")
            strip2 = spool.tile([128, 64, 130], sdt, name="strip2")
            strip3a = spool.tile([128, 64, 66], sdt, name="strip3a")
            strip3b = spool.tile([64, 64, 66], sdt, name="strip3b")
            xpooled1 = spool.tile([128, 64, 130], sdt, name="xpooled1")
            xpB1 = spool.tile([64, 64, 130], sdt, name="xpB1")
            h1X = spool.tile([64, 64, 130], sdt, name="h1X")
            xpooled2 = spool.tile([128, 64, 66], sdt, name="xpooled2")
            xpB2 = spool.tile([64, 64, 66], sdt, name="xpB2")
            h2buf = spool.tile([64, 64, 66], sdt, name="h2buf")
            xm = spool.tile([128, 2, 258], sdt, name="xm")
            qacc = spool.tile([128, 8], F32, name="qacc")
            qT = spool.tile([128, b_loc], F32, name="qT")
            xmd = dpool.tile([258, 258], sdt, name="xmd")

            for t in (strip1, strip2, strip3a, strip3b, xpooled1, xpooled2):
                nc.vector.memset(t, 0.0)
            nc.vector.memset(xm, 0.0)
            nc.vector.memset(strip1[0:1], 1.0)  # conv1 bias row (ones)
            zrow = cpool.tile([1, 2, 258], sdt, name="zrow")
            nc.vector.memset(zrow, 0.0)
            nc.sync.dma_start(  # xmd pad rows 0, 257 (cols padded per-write)
                out=dramap(xmd, 0, [[1, 1], [257 * 258, 2], [1, 258]]),
                in_=zrow)

            # ---------------- per-image pipeline helpers ----------------
            def load_x(img):
                t = xpool.tile([128, 2, 256], fft_in_dt, name="x_sb",
                               tag="x_sb")
                nc.gpsimd.dma_start(
                    out=t,
                    in_=dramap(x_in.ap(), img * 65536,
                               [[512, 128], [256, 2], [1, 256]]))
                return t

            def fft_step1(x_sb):
                """step1: yts[(nm, mt)] sbuf bf16 tiles [128(x), 256(u)]."""
                yts = {}
                for mt in range(2):
                    for nm, rt in (("re", "gre"), ("im", "gim")):
                        ps = fftps.tile([128, 256], F32, name="ps_yt",
                                        tag="fft")
                        n_mm = 2 * NSPLIT
                        i = 0
                        for kt in range(2):
                            for sp in range(NSPLIT):
                                nc.tensor.matmul(
                                    ps,
                                    x_sb[:, kt, mt * 128:(mt + 1) * 128],
                                    cs[rt][:, kt, sp, :],
                                    start=(i == 0), stop=(i == n_mm - 1))
                                i += 1
                        sb = wpool.tile([128, 256], fft_in_dt,
                                        name=f"yt{nm}{mt}", tag=f"yt{nm}{mt}")
                        nc.scalar.copy(sb, ps)
                        yts[(nm, mt)] = sb
                return yts

            def fft_step2_mag(yts):
                """step2 + magnitude -> xm [128, 2, 258] (padded cols)."""
                for mt in range(2):
                    zre = fftps.tile([128, 256], F32, name="ps_zre", tag="fft")
                    zim = fftps.tile([128, 256], F32, name="ps_zim", tag="fft")
                    for out_ps, combos in (
                        (zre, [("re", "gre"), ("im", "gimn")]),
                        (zim, [("re", "gim"), ("im", "gre")]),
                    ):
                        n_mm = 4 * NSPLIT
                        i = 0
                        for nm, rt in combos:
                            for kt in range(2):
                                for sp in range(NSPLIT):
                                    nc.tensor.matmul(
                                        out_ps,
                                        yts[(nm, kt)][:, mt * 128:(mt + 1) * 128],
                                        cs[rt][:, kt, sp, :],
                                        start=(i == 0), stop=(i == n_mm - 1))
                                    i += 1
                    t1 = wpool.tile([128, 256], F32, name="mag1", tag="mag1")
                    t2 = wpool.tile([128, 256], F32, name="mag2", tag="mag2")
                    nc.scalar.square(t1, zre)
                    nc.scalar.square(t2, zim)
                    nc.vector.tensor_add(t1, t1, t2)
                    nc.scalar.sqrt(mk(xm, 0, 1, 128, [[1, 256]], mt * 258 + 1),
                                   t1)

            dma_q = nc.sync if os.environ.get("K_SYNCQ") else nc.gpsimd

            def xm_to_dram_and_strips(hs_list=(0, 1)):
                # full 258-wide rows (pads included) -> contiguous-ish writes
                dma_q.dma_start(
                    out=dramap(xmd, 258,
                               [[258, 128], [128 * 258, 2], [1, 258]]),
                    in_=xm)
                for hs in hs_list:
                    for dx in range(3):
                        dma_q.dma_start(
                            out=mk(strip1, 1 + 6 * dx, 1, 6,
                                   [[256, 32], [1, 256]], hs * 8192),
                            in_=dramap(xmd, 33024 * hs + dx,
                                       [[258, 6], [1032, 32], [1, 256]]))

            def conv1_half(hs):
                for ch in range(16):
                    sg = 32 * hs + 2 * ch
                    ps = convps.tile([128, 512], F32, name="c1ps", tag="conv")
                    for sp in range(NSPLIT):
                        nc.tensor.matmul(
                            ps, cs["w1l"][:, 0, sp, :],
                            strip1[:, hs, 2 * ch:2 * ch + 2, :],
                            start=(sp == 0), stop=(sp == NSPLIT - 1))
                    # relu on ACT (bias is in K-row 18), x-pool on DVE
                    rt = wpool.tile([128, 2, 256], sdt, name="rt1", tag="rt1",
                                    bufs=3)
                    nc.scalar.activation(rt, ps, ActFn.Relu)
                    nc.vector.tensor_max(
                        mk(xpooled1, 0, 1, 128, [[130, 2], [1, 128]],
                           sg * 130 + 1),
                        mk(rt, 0, 1, 128, [[256, 2], [2, 128]], 0),
                        mk(rt, 0, 1, 128, [[256, 2], [2, 128]], 1))
                # per-half y-pool: align upper partitions, fold into h1X
                lo, n = hs * 32 * 130, 32 * 130
                nc.sync.dma_start(
                    out=mk(xpB1, 0, 1, 64, [[1, n]], lo),
                    in_=mk(xpooled1, 64, 1, 64, [[1, n]], lo))
                nc.vector.tensor_max(
                    mk(h1X, 0, 1, 64, [[1, n]], lo),
                    mk(xpooled1, 0, 1, 64, [[1, n]], lo),
                    mk(xpB1, 0, 1, 64, [[1, n]], lo))

            def fills2():
                # strip2 fills: slot s2 of j holds h1 row 2*s2+j-1;
                # h1X partitions 0..31 = even rows (slot=y/2), 32..63 = odd.
                for j, d0, ns, g, s0 in ((0, 1, 63, 32, 0), (1, 0, 64, 0, 0),
                                         (2, 0, 64, 32, 0), (3, 0, 63, 0, 1)):
                    nc.sync.dma_start(
                        out=mk(strip2, 32 * j, 1, 32, [[1, ns * 130]],
                               d0 * 130),
                        in_=mk(h1X, g, 1, 32, [[1, ns * 130]], s0 * 130))

            def conv2_half(half):
                for ch in range(8 * half, 8 * half + 8):
                    ps = convps.tile([128, 512], F32, name="c2ps", tag="conv")
                    i = 0
                    for dx in range(3):
                        for sp in range(NSPLIT):
                            nc.tensor.matmul(
                                ps, cs["w2l"][:, dx, sp, :],
                                mk(strip2, 0, 1, 128, [[130, 4], [1, 128]],
                                   4 * ch * 130 + dx),
                                start=(i == 0), stop=(i == 3 * NSPLIT - 1))
                            i += 1
                    rt2 = wpool.tile([128, 4, 128], sdt, name="rt2", tag="rt2",
                                     bufs=3)
                    nc.scalar.activation(rt2, ps, ActFn.Relu,
                                         bias=cs["cb2"][:, 0:1])
                    nc.vector.tensor_max(
                        mk(xpooled2, 0, 1, 128, [[66, 4], [1, 64]],
                           4 * ch * 66 + 1),
                        mk(rt2, 0, 1, 128, [[128, 4], [2, 64]], 0),
                        mk(rt2, 0, 1, 128, [[128, 4], [2, 64]], 1))
                # per-half align + fold into h2buf (strip3 fills happen
                # later, after the previous image's conv3 has consumed the
                # strips)
                lo, n = half * 32 * 66, 32 * 66
                nc.sync.dma_start(
                    out=mk(xpB2, 0, 1, 64, [[1, n]], lo),
                    in_=mk(xpooled2, 64, 1, 64, [[1, n]], lo))
                nc.vector.tensor_max(
                    mk(h2buf, 0, 1, 64, [[1, n]], lo),
                    mk(xpooled2, 0, 1, 64, [[1, n]], lo),
                    mk(xpB2, 0, 1, 64, [[1, n]], lo))

            def fills3():
                # strip3a: d=0 partitions hold row s-1, d=1 hold row s;
                # strip3b holds row s+1 (edge slots stay zero from init).
                for st, p0, d0, ns, s0 in (
                        (strip3a, 0, 1, 63, 0), (strip3a, 64, 0, 64, 0),
                        (strip3b, 0, 0, 63, 1)):
                    nc.sync.dma_start(
                        out=mk(st, p0, 1, 64, [[1, ns * 66]], d0 * 66),
                        in_=mk(h2buf, 0, 1, 64, [[1, ns * 66]], s0 * 66))

            def conv3_all(img):
                for ch in range(8):
                    ps = convps.tile([128, 512], F32, name="c3ps", tag="conv")
                    n_mm = 6 * NSPLIT
                    i = 0
                    for dx in range(3):
                        for w_, st3, pc in (("w3a", strip3a, 128),
                                            ("w3b", strip3b, 64)):
                            for sp in range(NSPLIT):
                                nc.tensor.matmul(
                                    ps, cs[w_][:, dx, sp, :],
                                    mk(st3, 0, 1, pc, [[66, 8], [1, 64]],
                                       8 * ch * 66 + dx),
                                    start=(i == 0), stop=(i == n_mm - 1))
                                i += 1
                    scr = wpool.tile([128, 512], F32, name="scr3", tag="scr3",
                                     bufs=2)
                    nc.scalar.activation(scr, ps, ActFn.Relu,
                                         bias=cs["cb3"][:, 0:1],
                                         accum_out=qacc[:, ch:ch + 1])
                nc.vector.reduce_sum(qT[:, img:img + 1], qacc, axis=AxX)

            # ---------------- software-pipelined image loop ----------------
            rep = int(os.environ.get("K_REP", "1"))
            loop_cm = tc.For_i(0, rep, 1) if rep > 1 else None
            if loop_cm is not None:
                loop_cm.__enter__()

            x_tiles = {0: load_x(0)}
            if b_loc > 1:
                x_tiles[1] = load_x(1)

            def load_misc_consts():
                for name, (shape, kind) in _const_specs().items():
                    if name not in crit:
                        nc.gpsimd.dma_start(out=cs[name],
                                            in_=const_d[name].ap())
            if os.environ.get("K_NOSKEW"):
                for i in range(b_loc):
                    yts = fft_step1(x_tiles[i])
                    fft_step2_mag(yts)
                    xm_to_dram_and_strips()
                    if i == 0:
                        load_misc_consts()
                    if i + 2 < b_loc:
                        x_tiles[i + 2] = load_x(i + 2)
                    conv1_half(0)
                    conv1_half(1)
                    fills2()
                    conv2_half(0)
                    conv2_half(1)
                    fills3()
                    conv3_all(i)
                    x_tiles.pop(i, None)
            else:
                # 2-deep skew: iteration i runs FFT(i+1), conv1/2(i), and
                # conv3(i-1), so every staging chain has a full phase of
                # matmuls to hide behind.
                yts = fft_step1(x_tiles[0])
                fft_step2_mag(yts)
                xm_to_dram_and_strips()
                load_misc_consts()
                for i in range(b_loc):
                    if i + 1 < b_loc:
                        yts = fft_step1(x_tiles[i + 1])
                    conv1_half(0)
                    conv1_half(1)
                    fills2()
                    if i + 1 < b_loc:
                        fft_step2_mag(yts)
                        xm_to_dram_and_strips()
                    if i + 2 < b_loc:
                        x_tiles[i + 2] = load_x(i + 2)
                    conv2_half(0)
                    conv2_half(1)
                    if i >= 1:
                        conv3_all(i - 1)
                    fills3()
                    x_tiles.pop(i, None)
                conv3_all(b_loc - 1)

            # ---------------- retrieval (fp32) ----------------
            # key normalization -> knT [128, 400] (fp32)
            knT = rpool.tile([128, 400], F32, name="knT")
            ksq = rpool.tile([128, 4, 128], F32, name="ksq")
            nc.vector.tensor_mul(ksq, cs["keys"], cs["keys"])
            kss = rpool.tile([128, 4], F32, name="kss")
            nc.vector.reduce_sum(kss, ksq, axis=AxX)
            knm = rpool.tile([128, 4], F32, name="knm")
            nc.scalar.sqrt(knm, kss)
            nc.vector.tensor_scalar_max(knm, knm, 1e-12)
            kri = rpool.tile([128, 4], F32, name="kri")
            nc.vector.reciprocal(kri, knm)
            knrm = rpool.tile([128, 4, 128], F32, name="knrm")
            for c in range(4):
                nc.vector.tensor_scalar_mul(
                    knrm[:, c, :], cs["keys"][:, c, :], kri[:, c:c + 1])
            for c in range(4):
                pc = 128 if c < 3 else 16
                tp = miscps.tile([128, 128], F32, name="tp_kn", tag="misc")
                nc.tensor.transpose(
                    tp[:, :pc], knrm[:pc, c, :], cs["ident"][:pc, :pc])
                nc.scalar.copy(knT[:, c * 128:c * 128 + pc], tp[:, :pc])

            if qdbg_d is not None:
                nc.sync.dma_start(out=qdbg_d.ap(), in_=qT)
            if hdbg_d is not None:
                # dump last-image staging tiles raw (bf16)
                for nm_, src_ in (("dxm", xm), ("dh1", h1X), ("dh2", h2buf),
                                  ("ds2", strip2)):
                    nc.sync.dma_start(out=hdbg_d[nm_].ap(), in_=src_)
            bl = b_loc
            simps = miscps.tile([bl, 400], F32, name="simps", tag="misc")
            nc.tensor.matmul(simps, qT, knT, start=True, stop=True)
            gram = miscps.tile([bl, bl], F32, name="gram", tag="misc")
            nc.tensor.matmul(gram, qT, qT, start=True, stop=True)
            gd = rpool.tile([bl, bl], F32, name="gd")
            nc.vector.tensor_mul(gd, gram, cs["ident"][:bl, :bl])
            q2 = rpool.tile([bl, 1], F32, name="q2")
            nc.vector.reduce_sum(q2, gd, axis=AxX)
            qn = rpool.tile([bl, 1], F32, name="qn")
            nc.scalar.sqrt(qn, q2)
            nc.vector.tensor_scalar_max(qn, qn, 1e-12)
            rq = rpool.tile([bl, 1], F32, name="rq")
            nc.vector.reciprocal(rq, qn)
            sim = rpool.tile([bl, 400], F32, name="sim")
            nc.vector.tensor_scalar_mul(sim, simps, rq[:, 0:1])

            cur = rpool.tile([bl, 400], F32, name="cur")
            nc.vector.tensor_copy(cur, sim)
            m1 = rpool.tile([bl, 1], F32, name="m1")
            nc.vector.reduce_max(m1, sim, axis=AxX)
            msk = rpool.tile([bl, 400], F32, name="msk")
            mk_ = m1
            for it in range(4):
                nc.vector.tensor_scalar(msk, cur, mk_[:, 0:1], None,
                                        op0=AluOp.is_ge)
                nc.vector.scalar_tensor_tensor(cur, msk, -1e30, cur,
                                               op0=AluOp.mult, op1=AluOp.add)
                nm_ = rpool.tile([bl, 1], F32, name=f"mk{it}")
                nc.vector.reduce_max(nm_, cur, axis=AxX)
                mk_ = nm_
            m5 = mk_
            nc.vector.tensor_scalar(msk, sim, m5[:, 0:1], None, op0=AluOp.is_ge)
            m1n = rpool.tile([bl, 1], F32, name="m1n")
            nc.vector.tensor_scalar_mul(m1n, m1, -1.0)
            es = rpool.tile([bl, 400], F32, name="es")
            nc.scalar.activation(es, sim, ActFn.Exp, bias=m1n[:, 0:1])
            ew = rpool.tile([bl, 400], F32, name="ew")
            nc.vector.tensor_mul(ew, es, msk)
            zs = rpool.tile([bl, 1], F32, name="zs")
            nc.vector.reduce_sum(zs, ew, axis=AxX)
            rz = rpool.tile([bl, 1], F32, name="rz")
            nc.vector.reciprocal(rz, zs)
            nc.vector.tensor_scalar_mul(ew, ew, rz[:, 0:1])

            eT = rpool.tile([128, 4, bl], F32, name="eT")
            for c in range(4):
                pc = 128 if c < 3 else 16
                tp = miscps.tile([128, bl], F32, name="tp_e", tag="misc")
                nc.tensor.transpose(tp[:pc, :], ew[:, c * 128:c * 128 + pc],
                                    cs["ident"][:bl, :bl])
                nc.scalar.copy(eT[:pc, c, :], tp[:pc, :])

            memps = miscps.tile([128, bl], F32, name="memps", tag="misc")
            for c in range(4):
                pc = 128 if c < 3 else 16
                nc.tensor.matmul(memps, cs["vals"][:pc, c, :], eT[:pc, c, :],
                                 start=(c == 0), stop=(c == 3))
            memT = rpool.tile([128, bl], F32, name="memT")
            nc.scalar.copy(memT, memps)

            h1T = rpool.tile([128, 2, bl], F32, name="h1T")
            for mt in range(2):
                ps = miscps.tile([128, bl], F32, name="d1ps", tag="misc")
                nc.tensor.matmul(ps, cs["w1d"][:, mt * 128:(mt + 1) * 128],
                                 memT, start=True, stop=True)
                nc.scalar.activation(h1T[:, mt, :], ps, ActFn.Relu,
                                     bias=cs["b1d"][:, mt:mt + 1])
            h2T = rpool.tile([128, 4, bl], F32, name="h2T")
            for mt in range(4):
                ps = miscps.tile([128, bl], F32, name="d2ps", tag="misc")
                for kt in range(2):
                    nc.tensor.matmul(ps, cs["w2d"][:, kt, mt, :], h1T[:, kt, :],
                                     start=(kt == 0), stop=(kt == 1))
                nc.scalar.activation(h2T[:, mt, :], ps, ActFn.Relu,
                                     bias=cs["b2d"][:, mt:mt + 1])
            ops = miscps.tile([bl, 16], F32, name="outps", tag="misc")
            for c in range(4):
                nc.tensor.matmul(ops, h2T[:, c, :], cs["w3d"][:, c, :],
                                 start=(c == 0), stop=False)
            nc.tensor.matmul(ops, cs["ones1"][:, :bl], cs["b3row"],
                             start=False, stop=True)
            out_sb = rpool.tile([bl, 16], F32, name="out_sb")
            nc.scalar.copy(out_sb, ops)
            nc.sync.dma_start(out=out_d.ap(), in_=out_sb)
            if loop_cm is not None:
                loop_cm.__exit__(None, None, None)

    nc.compile()
    return nc


# ---------------------------------------------------------------------------
# host entry
# ---------------------------------------------------------------------------
_NC_CACHE = {}


def _get_nc(b_loc):
    key = (b_loc, STAGE, os.environ.get("K_REP", "1"),
           os.environ.get("K_NOSKEW"), os.environ.get("K_SYNCQ"),
           os.environ.get("K_DBGQ"), os.environ.get("K_DBGH"))
    if key not in _NC_CACHE:
        _NC_CACHE[key] = build_nc(b_loc)
    return _NC_CACHE[key]


def _pack_x(x_shard):
    b = x_shard.shape[0]
    xr = np.ascontiguousarray(
        x_shard.reshape(b, 2, 128, 256).transpose(0, 2, 1, 3)).astype(np.float32)
    return xr.astype(_np_sdt())


def kernel(**inputs):
    x = np.asarray(inputs["x"], np.float32)
    # jnp.fft.fftshift also shifts the batch axis: output b uses x[(b+64)%128]
    xp = np.roll(x, -64, axis=0)
    consts = _host_consts(inputs)

    b_loc = B // N_CORES
    nc = _get_nc(b_loc)

    in_maps = []
    for c in range(N_CORES):
        m = dict(consts)
        m["x_in"] = _pack_x(xp[c * b_loc:(c + 1) * b_loc])
        in_maps.append(m)

    kwargs = {}
    if os.environ.get("K_TRACE"):
        kwargs["trace"] = True
    res = run_bass_kernel_spmd(nc, in_maps, core_ids=list(range(N_CORES)),
                               **kwargs)
    global LAST_RESULTS
    LAST_RESULTS = res
    out = np.concatenate([r["out"] for r in res.results], axis=0)
    return out.reshape(B, 1, 4, 4).astype(np.float32)


LAST_RESULTS = None


if __name__ == "__main__":
    build_nc(int(os.environ.get("K_BLOC", "1")))
    print("built ok")


# revision 39
# speedup vs baseline: 1.0094x; 1.0094x over previous
"""Trainium2 Bass kernel for nn_FFTMemAutoEncoderBranch (retrieval_knn).

Data-parallel over batch: 8 cores x 16 images, no cross-core communication.

Numerics: the problem's top-5 retrieval runs on near-identical queries (white
-noise FFT magnitudes), with 5th/6th similarity gaps down to 7e-6 -- so conv
weights and DFT matrices must act at ~fp32 fidelity while activations tolerate
bf16. Scheme (validated against the reference on HW, 0/128 top-5 flips):
  - activations/staging in bf16
  - every stationary operand (DFT matrices G, conv weights) is split
    W = hi + lo into two bf16 matmuls accumulating in fp32 PSUM
  - retrieval + decoder in fp32

Performance structure (v2): the v1 kernel ran image-major with serial staging,
leaving the PE idle ~80us/image (trace: 1.25ms of gaps in a 3.2ms span, HAM
re-throttling the PE to 1.2GHz for ~85% of the run) and moving ~100MB/core of
SBUF<->SBUF staging in 256-512B DMA descriptors. v2:
  - software-pipelines images with a one-iteration skew: PE order is
    [FFTs1(i+1) | conv1(i) | FFTs2(i+1) | conv2(i) | conv3(i)], so every
    staging chain (mag->DRAM bounce->strip1; pool->align->fold->strip fills)
    runs in the shadow of ~30-90us of matmuls from the neighboring image.
  - staging tiles use pitch-matched padded rows (130-wide for conv2 strips,
    66-wide for conv3) so each strip fill is a handful of multi-KB-contiguous
    descriptors instead of thousands of 256B ones.
  - conv1 folds bias into a 19th K-row (rhs row of ones) and fuses
    relu+x-pool into one DVE scalar_tensor_tensor reading PSUM directly.
  - pool/align/fold/fill chains run per half-image so conv(i) chunk k's rhs
    is staged while chunks k-1 of the same image still run.

Per image: FFT2 as DFT matmuls (z = G x G^T, G = roll(F,128,0)/16, fftshift+
ortho folded in; batch roll done on host), conv1 strips via a DRAM bounce of
the padded 258x258 magnitude image, conv2/conv3 strips via parity-split SBUF
copies, maxpools on DVE, conv3 relu accumulating into q via ACT accum_out.
Retrieval: fp32 sim, top-5 threshold via 5x(reduce_max+mask), masked stable
softmax, mem = values^T @ e^T; decoder collapsed to 3 dense matmuls.
"""

import os
import sys
import numpy as np

for _p in ("/opt/trn_rl_repo", "/root/.axon_site/_ro/trn_rl_repo"):
    if os.path.isdir(_p) and _p not in sys.path:
        sys.path.append(_p)

import concourse.bass as bass
import concourse.mybir as mybir
import concourse.tile as tile
from concourse import bacc
from concourse.bass_utils import run_bass_kernel_spmd

F32 = mybir.dt.float32
BF16 = mybir.dt.bfloat16

N_CORES = 8
B = 128
H = 256

# STAGE: "bf16" (hi/lo-split weights, bf16 activations) | "f32" (all fp32)
STAGE = os.environ.get("K_STAGE", "bf16")
NSPLIT = 2 if STAGE == "bf16" else 1

AluOp = mybir.AluOpType
ActFn = mybir.ActivationFunctionType
AxX = mybir.AxisListType.X


def _sdt():
    return BF16 if STAGE == "bf16" else F32


def _np_sdt():
    if STAGE == "bf16":
        import ml_dtypes
        return ml_dtypes.bfloat16
    return np.float32


# ---------------------------------------------------------------------------
# host-side constant construction
# ---------------------------------------------------------------------------
def _pack2(m):  # [256, N] -> [128, 2, N]
    return np.ascontiguousarray(m.reshape(2, 128, -1).transpose(1, 0, 2))


def _fft_consts():
    k = np.arange(H)
    F = np.exp(-2j * np.pi * np.outer(k, k) / H) / 16.0
    G = np.roll(F, H // 2, axis=0)
    GT = G.T.copy()
    out = {}
    for name, m in (("gre", GT.real), ("gim", GT.imag), ("gimn", -GT.imag)):
        m = m.astype(np.float32)
        if NSPLIT == 1:
            out[name] = _pack2(m)[:, :, None, :]  # [128, 2, 1, 256] f32
        else:
            sdt = _np_sdt()
            hi32 = m.astype(sdt).astype(np.float32)
            lo = (m - hi32).astype(sdt)
            hi = m.astype(sdt)
            # [128, 2, 2, 256]: (part-of-256-rows, half, split, col)
            out[name] = np.stack([_pack2(hi), _pack2(lo)], axis=2)
    return out


T1ORD = (0, 2, 1, 3)  # conv1 M block -> strip row offset t; so that y-pool
# pairs (t0,t1),(t2,t3) become max(partitions 0:64, partitions 64:128)


def _conv1_lhsT(we1, be1):
    # K = (dx, j) packed on partitions 1..18, p = 1 + dx*6 + j (dx-major so
    # each strip1 fill DMA writes a contiguous partition block); partition 0
    # is the bias row (strip1 partition 0 holds ones; engine-op partition
    # bases must be 32-aligned, so the ones memset needs base 0).
    out = np.zeros((19, 1, 128), np.float32)
    for dx in range(3):
        for m in range(128):
            t, co = T1ORD[m // 32], m % 32
            for j in range(6):
                if 0 <= j - t <= 2:
                    out[1 + dx * 6 + j, 0, m] = we1[co, 0, j - t, dx]
    for m in range(128):
        out[0, 0, m] = be1[m % 32]
    return out


def _conv2_lhsT(we2):
    # K layout j-major: k = j*32 + ci (each strip2 fill writes a contiguous
    # partition block)
    out = np.zeros((128, 3, 128), np.float32)
    for dx in range(3):
        for m in range(128):
            t, co = m // 64, m % 64
            for k in range(128):
                ci, j = k % 32, k // 32
                if 0 <= j - t <= 2:
                    out[k, dx, m] = we2[co, ci, j - t, dx]
    return out


def _conv3_lhsT(we3):
    # K layout d-major: k = d*64 + ci
    A = np.zeros((128, 3, 128), np.float32)
    Bm = np.zeros((64, 3, 128), np.float32)
    for dx in range(3):
        for k in range(128):
            ci, d = k % 64, k // 64
            A[k, dx, :] = we3[:, ci, d, dx]
        for ci in range(64):
            Bm[ci, dx, :] = we3[:, ci, 2, dx]
    return A, Bm


def _wsplit(w):
    """[P, D, N] -> [P, D, NSPLIT, N] staging dtype (hi, lo)."""
    sdt = _np_sdt()
    if NSPLIT == 1:
        return w[:, :, None, :].astype(np.float32)
    hi32 = w.astype(sdt).astype(np.float32)
    lo = (w - hi32).astype(sdt)
    return np.stack([w.astype(sdt), lo], axis=2)


def _decoder_mats(wd1, bd1, wd2, bd2, wd3, bd3):
    W1 = np.zeros((128, 256), np.float32)
    for c in range(64):
        for i in range(2):
            for j in range(2):
                W1[:, c * 4 + i * 2 + j] = wd1[:, c, i + 1, j + 1]
    b1 = np.repeat(bd1, 4).astype(np.float32)

    W2 = np.zeros((256, 512), np.float32)
    for c in range(64):
        for ii in range(2):
            for jj in range(2):
                f = c * 4 + ii * 2 + jj
                for c2 in range(32):
                    for y in range(4):
                        ky = y + 1 - 2 * ii
                        if not (0 <= ky <= 3):
                            continue
                        for x in range(4):
                            kx = x + 1 - 2 * jj
                            if 0 <= kx <= 3:
                                W2[f, c2 * 16 + y * 4 + x] = wd2[c, c2, ky, kx]
    b2 = np.repeat(bd2, 16).astype(np.float32)

    W3 = np.zeros((512, 16), np.float32)
    for c2 in range(32):
        for y in range(4):
            for x in range(4):
                g = c2 * 16 + y * 4 + x
                for oy in range(4):
                    ky = y - oy + 1
                    if not (0 <= ky <= 2):
                        continue
                    for ox in range(4):
                        kx = x - ox + 1
                        if 0 <= kx <= 2:
                            W3[g, oy * 4 + ox] = wd3[0, c2, ky, kx]
    b3 = np.full((16,), float(np.asarray(bd3).reshape(-1)[0]), np.float32)
    return W1, b1, W2, b2, W3, b3


def _host_consts(inputs):
    w3a, w3b = _conv3_lhsT(np.asarray(inputs["we3"], np.float32))
    W1, b1, W2, b2, W3, b3 = _decoder_mats(
        np.asarray(inputs["wd1"], np.float32), np.asarray(inputs["bd1"], np.float32),
        np.asarray(inputs["wd2"], np.float32), np.asarray(inputs["bd2"], np.float32),
        np.asarray(inputs["wd3"], np.float32), np.asarray(inputs["bd3"], np.float32))

    keys = np.asarray(inputs["keys"], np.float32)
    values = np.asarray(inputs["values"], np.float32)
    keys_p = np.ones((512, 128), np.float32)
    keys_p[:400] = keys
    values_p = np.zeros((512, 128), np.float32)
    values_p[:400] = values

    c = dict(_fft_consts())
    c.update({
        "w1l": _wsplit(_conv1_lhsT(np.asarray(inputs["we1"], np.float32),
                                   np.asarray(inputs["be1"], np.float32))),
        "w2l": _wsplit(_conv2_lhsT(np.asarray(inputs["we2"], np.float32))),
        "w3a": _wsplit(w3a), "w3b": _wsplit(w3b),
        "cb2": np.tile(np.asarray(inputs["be2"], np.float32), 2).reshape(128, 1),
        "cb3": np.asarray(inputs["be3"], np.float32).reshape(128, 1),
        "keys": np.ascontiguousarray(keys_p.reshape(4, 128, 128).transpose(1, 0, 2)),
        "vals": np.ascontiguousarray(values_p.reshape(4, 128, 128).transpose(1, 0, 2)),
        "ident": np.eye(128, dtype=np.float32),
        "w1d": W1,
        "w2d": np.ascontiguousarray(W2.reshape(2, 128, 4, 128).transpose(1, 0, 2, 3)),
        "w3d": np.ascontiguousarray(W3.reshape(4, 128, 16).transpose(1, 0, 2)),
        "b1d": np.ascontiguousarray(b1.reshape(2, 128).T),
        "b2d": np.ascontiguousarray(b2.reshape(4, 128).T),
        "b3row": b3.reshape(1, 16),
        "ones1": np.ones((1, 16), np.float32),
    })
    return c


def _const_specs():
    s = "stage"
    return {
        "gre": ([128, 2, NSPLIT, 256], s), "gim": ([128, 2, NSPLIT, 256], s),
        "gimn": ([128, 2, NSPLIT, 256], s),
        "w1l": ([19, 1, NSPLIT, 128], s), "w2l": ([128, 3, NSPLIT, 128], s),
        "w3a": ([128, 3, NSPLIT, 128], s), "w3b": ([64, 3, NSPLIT, 128], s),
        "cb2": ([128, 1], "f32"), "cb3": ([128, 1], "f32"),
        "keys": ([128, 4, 128], "f32"), "vals": ([128, 4, 128], "f32"),
        "ident": ([128, 128], "f32"),
        "w1d": ([128, 256], "f32"), "w2d": ([128, 2, 4, 128], "f32"),
        "w3d": ([128, 4, 16], "f32"),
        "b1d": ([128, 2], "f32"), "b2d": ([128, 4], "f32"),
        "b3row": ([1, 16], "f32"), "ones1": ([1, 16], "f32"),
    }


def mk(t, poff, pstep, pcount, fdims, foff=0):
    """Manual AP on tile t (element units; partition pitch from the tile AP)."""
    pitch = t.ap[0][0]
    dims = [[pstep * pitch, pcount]] + [list(d) for d in fdims]
    return bass.AP(t.tensor, t.offset + poff * pitch + foff, dims)


def dramap(t, off, dims):
    return bass.AP(t.tensor, t.offset + off, [list(d) for d in dims])


# ---------------------------------------------------------------------------
# kernel builder
# ---------------------------------------------------------------------------
def build_nc(b_loc=16):
    sdt = _sdt()
    fft_in_dt = BF16 if STAGE == "bf16" else F32
    nc = bacc.Bacc("TRN2", target_bir_lowering=False, debug=False)

    x_in = nc.dram_tensor("x_in", [b_loc, 128, 2, 256], fft_in_dt,
                          kind="ExternalInput")
    out_d = nc.dram_tensor("out", [b_loc, 16], F32, kind="ExternalOutput")
    qdbg_d = (nc.dram_tensor("qdbg", [128, b_loc], F32, kind="ExternalOutput")
              if os.environ.get("K_DBGQ") else None)
    hdbg_d = None
    if os.environ.get("K_DBGH"):
        _ddt = _sdt()
        hdbg_d = {
            "dxm": nc.dram_tensor("dxm", [128, 2, 258], _ddt,
                                  kind="ExternalOutput"),
            "dh1": nc.dram_tensor("dh1", [64, 64, 130], _ddt,
                                  kind="ExternalOutput"),
            "dh2": nc.dram_tensor("dh2", [64, 64, 66], _ddt,
                                  kind="ExternalOutput"),
            "ds2": nc.dram_tensor("ds2", [128, 64, 130], _ddt,
                                  kind="ExternalOutput"),
        }
    const_d = {}
    for name, (shape, kind) in _const_specs().items():
        dt_ = _sdt() if kind == "stage" else F32
        const_d[name] = nc.dram_tensor(name, shape, dt_, kind="ExternalInput")

    with tile.TileContext(nc) as tc:
        from contextlib import ExitStack
        with ExitStack() as ctx:
            cpool = ctx.enter_context(tc.tile_pool(name="consts", bufs=1))
            spool = ctx.enter_context(tc.tile_pool(name="stage", bufs=1))
            xpool = ctx.enter_context(tc.tile_pool(name="xin", bufs=3))
            wpool = ctx.enter_context(tc.tile_pool(name="work", bufs=2))
            rpool = ctx.enter_context(tc.tile_pool(name="ret", bufs=1))
            dpool = ctx.enter_context(tc.tile_pool(name="dram", bufs=1, space="DRAM"))
            fftps = ctx.enter_context(tc.tile_pool(name="fftps", bufs=3, space="PSUM"))
            convps = ctx.enter_context(tc.tile_pool(name="convps", bufs=3, space="PSUM"))
            miscps = ctx.enter_context(tc.tile_pool(name="miscps", bufs=2, space="PSUM"))

            # critical-path consts (FFT G matrices + conv1 weights) load
            # first on the sync queue; everything else goes on the gpsimd
            # queue so image-0's FFT isn't stuck behind ~1MB of decoder
            # weights.
            crit = ("gre", "gim", "gimn", "w1l")
            cs = {}
            for name, (shape, kind) in _const_specs().items():
                dt_ = _sdt() if kind == "stage" else F32
                t = cpool.tile(shape, dt_, name=f"c_{name}")
                if name in crit:
                    nc.sync.dma_start(out=t, in_=const_d[name].ap())
                cs[name] = t

            # fixed stage buffers (all single-buffered; WAR deps order reuse)
            strip1 = spool.tile([19, 2, 32, 256], sdt, name="strip1")
            strip2 = spool.tile([128, 64, 130], sdt, name="strip2")
            strip3a = spool.tile([128, 64, 66], sdt, name="strip3a")
            strip3b = spool.tile([64, 64, 66], sdt, name="strip3b")
            xpooled1 = spool.tile([128, 64, 130], sdt, name="xpooled1")
            xpB1 = spool.tile([64, 64, 130], sdt, name="xpB1")
            h1X = spool.tile([64, 64, 130], sdt, name="h1X")
            xpooled2 = spool.tile([128, 64, 66], sdt, name="xpooled2")
            xpB2 = spool.tile([64, 64, 66], sdt, name="xpB2")
            h2buf = spool.tile([64, 64, 66], sdt, name="h2buf")
            xm = spool.tile([128, 2, 258], sdt, name="xm")
            qacc = spool.tile([128, 8], F32, name="qacc")
            qT = spool.tile([128, b_loc], F32, name="qT")
            xmd = dpool.tile([258, 258], sdt, name="xmd")

            for t in (strip1, strip2, strip3a, strip3b, xpooled1, xpooled2):
                nc.vector.memset(t, 0.0)
            nc.vector.memset(xm, 0.0)
            nc.vector.memset(strip1[0:1], 1.0)  # conv1 bias row (ones)
            zrow = cpool.tile([1, 2, 258], sdt, name="zrow")
            nc.vector.memset(zrow, 0.0)
            nc.sync.dma_start(  # xmd pad rows 0, 257 (cols padded per-write)
                out=dramap(xmd, 0, [[1, 1], [257 * 258, 2], [1, 258]]),
                in_=zrow)

            # ---------------- per-image pipeline helpers ----------------
            def load_x(img):
                t = xpool.tile([128, 2, 256], fft_in_dt, name="x_sb",
                               tag="x_sb")
                nc.gpsimd.dma_start(
                    out=t,
                    in_=dramap(x_in.ap(), img * 65536,
                               [[512, 128], [256, 2], [1, 256]]))
                return t

            def fft_step1(x_sb):
                """step1: yts[(nm, mt)] sbuf bf16 tiles [128(x), 256(u)]."""
                yts = {}
                for mt in range(2):
                    for nm, rt in (("re", "gre"), ("im", "gim")):
                        ps = fftps.tile([128, 256], F32, name="ps_yt",
                                        tag="fft")
                        n_mm = 2 * NSPLIT
                        i = 0
                        for kt in range(2):
                            for sp in range(NSPLIT):
                                nc.tensor.matmul(
                                    ps,
                                    x_sb[:, kt, mt * 128:(mt + 1) * 128],
                                    cs[rt][:, kt, sp, :],
                                    start=(i == 0), stop=(i == n_mm - 1))
                                i += 1
                        sb = wpool.tile([128, 256], fft_in_dt,
                                        name=f"yt{nm}{mt}", tag=f"yt{nm}{mt}")
                        nc.scalar.copy(sb, ps)
                        yts[(nm, mt)] = sb
                return yts

            def fft_step2_mag(yts):
                """step2 + magnitude -> xm [128, 2, 258] (padded cols)."""
                for mt in range(2):
                    zre = fftps.tile([128, 256], F32, name="ps_zre", tag="fft")
                    zim = fftps.tile([128, 256], F32, name="ps_zim", tag="fft")
                    for out_ps, combos in (
                        (zre, [("re", "gre"), ("im", "gimn")]),
                        (zim, [("re", "gim"), ("im", "gre")]),
                    ):
                        n_mm = 4 * NSPLIT
                        i = 0
                        for nm, rt in combos:
                            for kt in range(2):
                                for sp in range(NSPLIT):
                                    nc.tensor.matmul(
                                        out_ps,
                                        yts[(nm, kt)][:, mt * 128:(mt + 1) * 128],
                                        cs[rt][:, kt, sp, :],
                                        start=(i == 0), stop=(i == n_mm - 1))
                                    i += 1
                    t1 = wpool.tile([128, 256], F32, name="mag1", tag="mag1")
                    t2 = wpool.tile([128, 256], F32, name="mag2", tag="mag2")
                    nc.scalar.square(t1, zre)
                    nc.scalar.square(t2, zim)
                    nc.vector.tensor_add(t1, t1, t2)
                    nc.scalar.sqrt(mk(xm, 0, 1, 128, [[1, 256]], mt * 258 + 1),
                                   t1)

            dma_q = nc.sync if os.environ.get("K_SYNCQ") else nc.gpsimd

            def xm_to_dram_and_strips(hs_list=(0, 1)):
                # full 258-wide rows (pads included) -> contiguous-ish writes
                dma_q.dma_start(
                    out=dramap(xmd, 258,
                               [[258, 128], [128 * 258, 2], [1, 258]]),
                    in_=xm)
                for hs in hs_list:
                    for dx in range(3):
                        dma_q.dma_start(
                            out=mk(strip1, 1 + 6 * dx, 1, 6,
                                   [[256, 32], [1, 256]], hs * 8192),
                            in_=dramap(xmd, 33024 * hs + dx,
                                       [[258, 6], [1032, 32], [1, 256]]))

            def conv1_half(hs):
                for ch in range(16):
                    sg = 32 * hs + 2 * ch
                    ps = convps.tile([128, 512], F32, name="c1ps", tag="conv")
                    for sp in range(NSPLIT):
                        nc.tensor.matmul(
                            ps, cs["w1l"][:, 0, sp, :],
                            strip1[:, hs, 2 * ch:2 * ch + 2, :],
                            start=(sp == 0), stop=(sp == NSPLIT - 1))
                    # relu on ACT (bias is in K-row 18), x-pool on DVE
                    rt = wpool.tile([128, 2, 256], sdt, name="rt1", tag="rt1",
                                    bufs=3)
                    nc.scalar.activation(rt, ps, ActFn.Relu)
                    nc.vector.tensor_max(
                        mk(xpooled1, 0, 1, 128, [[130, 2], [1, 128]],
                           sg * 130 + 1),
                        mk(rt, 0, 1, 128, [[256, 2], [2, 128]], 0),
                        mk(rt, 0, 1, 128, [[256, 2], [2, 128]], 1))
                # per-half y-pool: align upper partitions, fold into h1X
                lo, n = hs * 32 * 130, 32 * 130
                nc.sync.dma_start(
                    out=mk(xpB1, 0, 1, 64, [[1, n]], lo),
                    in_=mk(xpooled1, 64, 1, 64, [[1, n]], lo))
                nc.vector.tensor_max(
                    mk(h1X, 0, 1, 64, [[1, n]], lo),
                    mk(xpooled1, 0, 1, 64, [[1, n]], lo),
                    mk(xpB1, 0, 1, 64, [[1, n]], lo))

            def fills2():
                # strip2 fills: slot s2 of j holds h1 row 2*s2+j-1;
                # h1X partitions 0..31 = even rows (slot=y/2), 32..63 = odd.
                for j, d0, ns, g, s0 in ((0, 1, 63, 32, 0), (1, 0, 64, 0, 0),
                                         (2, 0, 64, 32, 0), (3, 0, 63, 0, 1)):
                    nc.sync.dma_start(
                        out=mk(strip2, 32 * j, 1, 32, [[1, ns * 130]],
                               d0 * 130),
                        in_=mk(h1X, g, 1, 32, [[1, ns * 130]], s0 * 130))

            def conv2_half(half):
                for ch in range(8 * half, 8 * half + 8):
                    ps = convps.tile([128, 512], F32, name="c2ps", tag="conv")
                    i = 0
                    for dx in range(3):
                        for sp in range(NSPLIT):
                            nc.tensor.matmul(
                                ps, cs["w2l"][:, dx, sp, :],
                                mk(strip2, 0, 1, 128, [[130, 4], [1, 128]],
                                   4 * ch * 130 + dx),
                                start=(i == 0), stop=(i == 3 * NSPLIT - 1))
                            i += 1
                    rt2 = wpool.tile([128, 4, 128], sdt, name="rt2", tag="rt2",
                                     bufs=3)
                    nc.scalar.activation(rt2, ps, ActFn.Relu,
                                         bias=cs["cb2"][:, 0:1])
                    nc.vector.tensor_max(
                        mk(xpooled2, 0, 1, 128, [[66, 4], [1, 64]],
                           4 * ch * 66 + 1),
                        mk(rt2, 0, 1, 128, [[128, 4], [2, 64]], 0),
                        mk(rt2, 0, 1, 128, [[128, 4], [2, 64]], 1))
                # per-half align + fold into h2buf (strip3 fills happen
                # later, after the previous image's conv3 has consumed the
                # strips)
                lo, n = half * 32 * 66, 32 * 66
                nc.sync.dma_start(
                    out=mk(xpB2, 0, 1, 64, [[1, n]], lo),
                    in_=mk(xpooled2, 64, 1, 64, [[1, n]], lo))
                nc.vector.tensor_max(
                    mk(h2buf, 0, 1, 64, [[1, n]], lo),
                    mk(xpooled2, 0, 1, 64, [[1, n]], lo),
                    mk(xpB2, 0, 1, 64, [[1, n]], lo))

            def fills3():
                # strip3a: d=0 partitions hold row s-1, d=1 hold row s;
                # strip3b holds row s+1 (edge slots stay zero from init).
                for st, p0, d0, ns, s0 in (
                        (strip3a, 0, 1, 63, 0), (strip3a, 64, 0, 64, 0),
                        (strip3b, 0, 0, 63, 1)):
                    nc.sync.dma_start(
                        out=mk(st, p0, 1, 64, [[1, ns * 66]], d0 * 66),
                        in_=mk(h2buf, 0, 1, 64, [[1, ns * 66]], s0 * 66))

            def conv3_all(img):
                for ch in range(8):
                    ps = convps.tile([128, 512], F32, name="c3ps", tag="conv")
                    n_mm = 6 * NSPLIT
                    i = 0
                    for dx in range(3):
                        for w_, st3, pc in (("w3a", strip3a, 128),
                                            ("w3b", strip3b, 64)):
                            for sp in range(NSPLIT):
                                nc.tensor.matmul(
                                    ps, cs[w_][:, dx, sp, :],
                                    mk(st3, 0, 1, pc, [[66, 8], [1, 64]],
                                       8 * ch * 66 + dx),
                                    start=(i == 0), stop=(i == n_mm - 1))
                                i += 1
                    scr = wpool.tile([128, 512], F32, name="scr3", tag="scr3",
                                     bufs=2)
                    nc.scalar.activation(scr, ps, ActFn.Relu,
                                         bias=cs["cb3"][:, 0:1],
                                         accum_out=qacc[:, ch:ch + 1])
                nc.vector.reduce_sum(qT[:, img:img + 1], qacc, axis=AxX)

            # ---------------- software-pipelined image loop ----------------
            rep = int(os.environ.get("K_REP", "1"))
            loop_cm = tc.For_i(0, rep, 1) if rep > 1 else None
            if loop_cm is not None:
                loop_cm.__enter__()

            x_tiles = {0: load_x(0)}
            if b_loc > 1:
                x_tiles[1] = load_x(1)

            def load_misc_consts():
                for name, (shape, kind) in _const_specs().items():
                    if name not in crit:
                        nc.gpsimd.dma_start(out=cs[name],
                                            in_=const_d[name].ap())
            if os.environ.get("K_NOSKEW"):
                for i in range(b_loc):
                    yts = fft_step1(x_tiles[i])
                    fft_step2_mag(yts)
                    xm_to_dram_and_strips()
                    if i == 0:
                        load_misc_consts()
                    if i + 2 < b_loc:
                        x_tiles[i + 2] = load_x(i + 2)
                    conv1_half(0)
                    conv1_half(1)
                    fills2()
                    conv2_half(0)
                    conv2_half(1)
                    fills3()
                    conv3_all(i)
                    x_tiles.pop(i, None)
            else:
                # 3-deep skew: iteration i runs FFT(i+1), conv1(i),
                # conv2(i-1) and conv3(i-2), so every staging chain has at
                # least a full conv phase of matmuls to hide behind.
                yts = fft_step1(x_tiles[0])
                fft_step2_mag(yts)
                xm_to_dram_and_strips()
                load_misc_consts()
                for i in range(b_loc):
                    if i + 1 < b_loc:
                        yts = fft_step1(x_tiles[i + 1])
                    conv1_half(0)
                    conv1_half(1)
                    if i + 1 < b_loc:
                        fft_step2_mag(yts)
                        xm_to_dram_and_strips()
                    if i + 2 < b_loc:
                        x_tiles[i + 2] = load_x(i + 2)
                    if i >= 1:
                        conv2_half(0)
                        conv2_half(1)
                    fills2()
                    if i >= 2:
                        conv3_all(i - 2)
                    if i >= 1:
                        fills3()
                    x_tiles.pop(i, None)
                conv2_half(0)
                conv2_half(1)
                if b_loc >= 2:
                    conv3_all(b_loc - 2)
                fills3()
                conv3_all(b_loc - 1)

            # ---------------- retrieval (fp32) ----------------
            # key normalization -> knT [128, 400] (fp32)
            knT = rpool.tile([128, 400], F32, name="knT")
            ksq = rpool.tile([128, 4, 128], F32, name="ksq")
            nc.vector.tensor_mul(ksq, cs["keys"], cs["keys"])
            kss = rpool.tile([128, 4], F32, name="kss")
            nc.vector.reduce_sum(kss, ksq, axis=AxX)
            knm = rpool.tile([128, 4], F32, name="knm")
            nc.scalar.sqrt(knm, kss)
            nc.vector.tensor_scalar_max(knm, knm, 1e-12)
            kri = rpool.tile([128, 4], F32, name="kri")
            nc.vector.reciprocal(kri, knm)
            knrm = rpool.tile([128, 4, 128], F32, name="knrm")
            for c in range(4):
                nc.vector.tensor_scalar_mul(
                    knrm[:, c, :], cs["keys"][:, c, :], kri[:, c:c + 1])
            for c in range(4):
                pc = 128 if c < 3 else 16
                tp = miscps.tile([128, 128], F32, name="tp_kn", tag="misc")
                nc.tensor.transpose(
                    tp[:, :pc], knrm[:pc, c, :], cs["ident"][:pc, :pc])
                nc.scalar.copy(knT[:, c * 128:c * 128 + pc], tp[:, :pc])

            if qdbg_d is not None:
                nc.sync.dma_start(out=qdbg_d.ap(), in_=qT)
            if hdbg_d is not None:
                # dump last-image staging tiles raw (bf16)
                for nm_, src_ in (("dxm", xm), ("dh1", h1X), ("dh2", h2buf),
                                  ("ds2", strip2)):
                    nc.sync.dma_start(out=hdbg_d[nm_].ap(), in_=src_)
            bl = b_loc
            simps = miscps.tile([bl, 400], F32, name="simps", tag="misc")
            nc.tensor.matmul(simps, qT, knT, start=True, stop=True)
            gram = miscps.tile([bl, bl], F32, name="gram", tag="misc")
            nc.tensor.matmul(gram, qT, qT, start=True, stop=True)
            gd = rpool.tile([bl, bl], F32, name="gd")
            nc.vector.tensor_mul(gd, gram, cs["ident"][:bl, :bl])
            q2 = rpool.tile([bl, 1], F32, name="q2")
            nc.vector.reduce_sum(q2, gd, axis=AxX)
            qn = rpool.tile([bl, 1], F32, name="qn")
            nc.scalar.sqrt(qn, q2)
            nc.vector.tensor_scalar_max(qn, qn, 1e-12)
            rq = rpool.tile([bl, 1], F32, name="rq")
            nc.vector.reciprocal(rq, qn)
            sim = rpool.tile([bl, 400], F32, name="sim")
            nc.vector.tensor_scalar_mul(sim, simps, rq[:, 0:1])

            cur = rpool.tile([bl, 400], F32, name="cur")
            nc.vector.tensor_copy(cur, sim)
            m1 = rpool.tile([bl, 1], F32, name="m1")
            nc.vector.reduce_max(m1, sim, axis=AxX)
            msk = rpool.tile([bl, 400], F32, name="msk")
            mk_ = m1
            for it in range(4):
                nc.vector.tensor_scalar(msk, cur, mk_[:, 0:1], None,
                                        op0=AluOp.is_ge)
                nc.vector.scalar_tensor_tensor(cur, msk, -1e30, cur,
                                               op0=AluOp.mult, op1=AluOp.add)
                nm_ = rpool.tile([bl, 1], F32, name=f"mk{it}")
                nc.vector.reduce_max(nm_, cur, axis=AxX)
                mk_ = nm_
            m5 = mk_
            nc.vector.tensor_scalar(msk, sim, m5[:, 0:1], None, op0=AluOp.is_ge)
            m1n = rpool.tile([bl, 1], F32, name="m1n")
            nc.vector.tensor_scalar_mul(m1n, m1, -1.0)
            es = rpool.tile([bl, 400], F32, name="es")
            nc.scalar.activation(es, sim, ActFn.Exp, bias=m1n[:, 0:1])
            ew = rpool.tile([bl, 400], F32, name="ew")
            nc.vector.tensor_mul(ew, es, msk)
            zs = rpool.tile([bl, 1], F32, name="zs")
            nc.vector.reduce_sum(zs, ew, axis=AxX)
            rz = rpool.tile([bl, 1], F32, name="rz")
            nc.vector.reciprocal(rz, zs)
            nc.vector.tensor_scalar_mul(ew, ew, rz[:, 0:1])

            eT = rpool.tile([128, 4, bl], F32, name="eT")
            for c in range(4):
                pc = 128 if c < 3 else 16
                tp = miscps.tile([128, bl], F32, name="tp_e", tag="misc")
                nc.tensor.transpose(tp[:pc, :], ew[:, c * 128:c * 128 + pc],
                                    cs["ident"][:bl, :bl])
                nc.scalar.copy(eT[:pc, c, :], tp[:pc, :])

            memps = miscps.tile([128, bl], F32, name="memps", tag="misc")
            for c in range(4):
                pc = 128 if c < 3 else 16
                nc.tensor.matmul(memps, cs["vals"][:pc, c, :], eT[:pc, c, :],
                                 start=(c == 0), stop=(c == 3))
            memT = rpool.tile([128, bl], F32, name="memT")
            nc.scalar.copy(memT, memps)

            h1T = rpool.tile([128, 2, bl], F32, name="h1T")
            for mt in range(2):
                ps = miscps.tile([128, bl], F32, name="d1ps", tag="misc")
                nc.tensor.matmul(ps, cs["w1d"][:, mt * 128:(mt + 1) * 128],
                                 memT, start=True, stop=True)
                nc.scalar.activation(h1T[:, mt, :], ps, ActFn.Relu,
                                     bias=cs["b1d"][:, mt:mt + 1])
            h2T = rpool.tile([128, 4, bl], F32, name="h2T")
            for mt in range(4):
                ps = miscps.tile([128, bl], F32, name="d2ps", tag="misc")
                for kt in range(2):
                    nc.tensor.matmul(ps, cs["w2d"][:, kt, mt, :], h1T[:, kt, :],
                                     start=(kt == 0), stop=(kt == 1))
                nc.scalar.activation(h2T[:, mt, :], ps, ActFn.Relu,
                                     bias=cs["b2d"][:, mt:mt + 1])
            ops = miscps.tile([bl, 16], F32, name="outps", tag="misc")
            for c in range(4):
                nc.tensor.matmul(ops, h2T[:, c, :], cs["w3d"][:, c, :],
                                 start=(c == 0), stop=False)
            nc.tensor.matmul(ops, cs["ones1"][:, :bl], cs["b3row"],
                             start=False, stop=True)
            out_sb = rpool.tile([bl, 16], F32, name="out_sb")
            nc.scalar.copy(out_sb, ops)
            nc.sync.dma_start(out=out_d.ap(), in_=out_sb)
            if loop_cm is not None:
                loop_cm.__exit__(None, None, None)

    nc.compile()
    return nc


# ---------------------------------------------------------------------------
# host entry
# ---------------------------------------------------------------------------
_NC_CACHE = {}


def _get_nc(b_loc):
    key = (b_loc, STAGE, os.environ.get("K_REP", "1"),
           os.environ.get("K_NOSKEW"), os.environ.get("K_SYNCQ"),
           os.environ.get("K_DBGQ"), os.environ.get("K_DBGH"))
    if key not in _NC_CACHE:
        _NC_CACHE[key] = build_nc(b_loc)
    return _NC_CACHE[key]


def _pack_x(x_shard):
    b = x_shard.shape[0]
    xr = np.ascontiguousarray(
        x_shard.reshape(b, 2, 128, 256).transpose(0, 2, 1, 3)).astype(np.float32)
    return xr.astype(_np_sdt())


def kernel(**inputs):
    x = np.asarray(inputs["x"], np.float32)
    # jnp.fft.fftshift also shifts the batch axis: output b uses x[(b+64)%128]
    xp = np.roll(x, -64, axis=0)
    consts = _host_consts(inputs)

    b_loc = B // N_CORES
    nc = _get_nc(b_loc)

    in_maps = []
    for c in range(N_CORES):
        m = dict(consts)
        m["x_in"] = _pack_x(xp[c * b_loc:(c + 1) * b_loc])
        in_maps.append(m)

    kwargs = {}
    if os.environ.get("K_TRACE"):
        kwargs["trace"] = True
    res = run_bass_kernel_spmd(nc, in_maps, core_ids=list(range(N_CORES)),
                               **kwargs)
    global LAST_RESULTS
    LAST_RESULTS = res
    out = np.concatenate([r["out"] for r in res.results], axis=0)
    return out.reshape(B, 1, 4, 4).astype(np.float32)


LAST_RESULTS = None


if __name__ == "__main__":
    build_nc(int(os.environ.get("K_BLOC", "1")))
    print("built ok")


# revision 40
# speedup vs baseline: 1.2934x; 1.2814x over previous
"""Trainium2 Bass kernel for nn_FFTMemAutoEncoderBranch (retrieval_knn).

Data-parallel over batch: 8 cores x 16 images, no cross-core communication.

Numerics: the problem's top-5 retrieval runs on near-identical queries (white
-noise FFT magnitudes), with 5th/6th similarity gaps down to 7e-6 -- so conv
weights and DFT matrices must act at ~fp32 fidelity while activations tolerate
bf16. Scheme (validated against the reference on HW, 0/128 top-5 flips):
  - activations/staging in bf16
  - every stationary operand (DFT matrices G, conv weights) is split
    W = hi + lo into two bf16 matmuls accumulating in fp32 PSUM
  - retrieval + decoder in fp32

Performance structure (v2): the v1 kernel ran image-major with serial staging,
leaving the PE idle ~80us/image (trace: 1.25ms of gaps in a 3.2ms span, HAM
re-throttling the PE to 1.2GHz for ~85% of the run) and moving ~100MB/core of
SBUF<->SBUF staging in 256-512B DMA descriptors. v2:
  - software-pipelines images with a one-iteration skew: PE order is
    [FFTs1(i+1) | conv1(i) | FFTs2(i+1) | conv2(i) | conv3(i)], so every
    staging chain (mag->DRAM bounce->strip1; pool->align->fold->strip fills)
    runs in the shadow of ~30-90us of matmuls from the neighboring image.
  - staging tiles use pitch-matched padded rows (130-wide for conv2 strips,
    66-wide for conv3) so each strip fill is a handful of multi-KB-contiguous
    descriptors instead of thousands of 256B ones.
  - conv1 folds bias into a 19th K-row (rhs row of ones) and fuses
    relu+x-pool into one DVE scalar_tensor_tensor reading PSUM directly.
  - pool/align/fold/fill chains run per half-image so conv(i) chunk k's rhs
    is staged while chunks k-1 of the same image still run.

Per image: FFT2 as DFT matmuls (z = G x G^T, G = roll(F,128,0)/16, fftshift+
ortho folded in; batch roll done on host), conv1 strips via a DRAM bounce of
the padded 258x258 magnitude image, conv2/conv3 strips via parity-split SBUF
copies, maxpools on DVE, conv3 relu accumulating into q via ACT accum_out.
Retrieval: fp32 sim, top-5 threshold via 5x(reduce_max+mask), masked stable
softmax, mem = values^T @ e^T; decoder collapsed to 3 dense matmuls.
"""

import os
import sys
import numpy as np

for _p in ("/opt/trn_rl_repo", "/root/.axon_site/_ro/trn_rl_repo"):
    if os.path.isdir(_p) and _p not in sys.path:
        sys.path.append(_p)

import concourse.bass as bass
import concourse.mybir as mybir
import concourse.tile as tile
from concourse import bacc
from concourse.bass_utils import run_bass_kernel_spmd

F32 = mybir.dt.float32
BF16 = mybir.dt.bfloat16

N_CORES = 8
B = 128
H = 256

# STAGE: "bf16" (hi/lo-split weights, bf16 activations) | "f32" (all fp32)
STAGE = os.environ.get("K_STAGE", "bf16")
NSPLIT = 2 if STAGE == "bf16" else 1

AluOp = mybir.AluOpType
ActFn = mybir.ActivationFunctionType
AxX = mybir.AxisListType.X


def _sdt():
    return BF16 if STAGE == "bf16" else F32


def _np_sdt():
    if STAGE == "bf16":
        import ml_dtypes
        return ml_dtypes.bfloat16
    return np.float32


# ---------------------------------------------------------------------------
# host-side constant construction
# ---------------------------------------------------------------------------
def _pack2(m):  # [256, N] -> [128, 2, N]
    return np.ascontiguousarray(m.reshape(2, 128, -1).transpose(1, 0, 2))


def _fft_consts():
    k = np.arange(H)
    F = np.exp(-2j * np.pi * np.outer(k, k) / H) / 16.0
    G = np.roll(F, H // 2, axis=0)
    GT = G.T.copy()
    out = {}
    for name, m in (("gre", GT.real), ("gim", GT.imag), ("gimn", -GT.imag)):
        m = m.astype(np.float32)
        if NSPLIT == 1:
            out[name] = _pack2(m)[:, :, None, :]  # [128, 2, 1, 256] f32
        else:
            sdt = _np_sdt()
            hi32 = m.astype(sdt).astype(np.float32)
            lo = (m - hi32).astype(sdt)
            hi = m.astype(sdt)
            # [128, 2, 2, 256]: (part-of-256-rows, half, split, col)
            out[name] = np.stack([_pack2(hi), _pack2(lo)], axis=2)
    return out


T1ORD = (0, 2, 1, 3)  # conv1 M block -> strip row offset t; so that y-pool
# pairs (t0,t1),(t2,t3) become max(partitions 0:64, partitions 64:128)


def _conv1_lhsT(we1, be1):
    # K = (dx, j) packed on partitions 1..18, p = 1 + dx*6 + j (dx-major so
    # each strip1 fill DMA writes a contiguous partition block); partition 0
    # is the bias row (strip1 partition 0 holds ones; engine-op partition
    # bases must be 32-aligned, so the ones memset needs base 0).
    out = np.zeros((128, 1, 128), np.float32)
    for dx in range(3):
        for m in range(128):
            t, co = T1ORD[m // 32], m % 32
            for j in range(6):
                if 0 <= j - t <= 2:
                    out[1 + dx * 6 + j, 0, m] = we1[co, 0, j - t, dx]
    for m in range(128):
        out[0, 0, m] = be1[m % 32]
    return out


def _conv2_lhsT(we2):
    # K layout j-major: k = j*32 + ci (each strip2 fill writes a contiguous
    # partition block)
    out = np.zeros((128, 3, 128), np.float32)
    for dx in range(3):
        for m in range(128):
            t, co = m // 64, m % 64
            for k in range(128):
                ci, j = k % 32, k // 32
                if 0 <= j - t <= 2:
                    out[k, dx, m] = we2[co, ci, j - t, dx]
    return out


def _conv3_lhsT(we3):
    # K layout d-major: k = d*64 + ci
    A = np.zeros((128, 3, 128), np.float32)
    Bm = np.zeros((64, 3, 128), np.float32)
    for dx in range(3):
        for k in range(128):
            ci, d = k % 64, k // 64
            A[k, dx, :] = we3[:, ci, d, dx]
        for ci in range(64):
            Bm[ci, dx, :] = we3[:, ci, 2, dx]
    return A, Bm


def _wsplit(w):
    """[P, D, N] -> [P, D, NSPLIT, N] staging dtype (hi, lo)."""
    sdt = _np_sdt()
    if NSPLIT == 1:
        return w[:, :, None, :].astype(np.float32)
    hi32 = w.astype(sdt).astype(np.float32)
    lo = (w - hi32).astype(sdt)
    return np.stack([w.astype(sdt), lo], axis=2)


def _decoder_mats(wd1, bd1, wd2, bd2, wd3, bd3):
    W1 = np.zeros((128, 256), np.float32)
    for c in range(64):
        for i in range(2):
            for j in range(2):
                W1[:, c * 4 + i * 2 + j] = wd1[:, c, i + 1, j + 1]
    b1 = np.repeat(bd1, 4).astype(np.float32)

    W2 = np.zeros((256, 512), np.float32)
    for c in range(64):
        for ii in range(2):
            for jj in range(2):
                f = c * 4 + ii * 2 + jj
                for c2 in range(32):
                    for y in range(4):
                        ky = y + 1 - 2 * ii
                        if not (0 <= ky <= 3):
                            continue
                        for x in range(4):
                            kx = x + 1 - 2 * jj
                            if 0 <= kx <= 3:
                                W2[f, c2 * 16 + y * 4 + x] = wd2[c, c2, ky, kx]
    b2 = np.repeat(bd2, 16).astype(np.float32)

    W3 = np.zeros((512, 16), np.float32)
    for c2 in range(32):
        for y in range(4):
            for x in range(4):
                g = c2 * 16 + y * 4 + x
                for oy in range(4):
                    ky = y - oy + 1
                    if not (0 <= ky <= 2):
                        continue
                    for ox in range(4):
                        kx = x - ox + 1
                        if 0 <= kx <= 2:
                            W3[g, oy * 4 + ox] = wd3[0, c2, ky, kx]
    b3 = np.full((16,), float(np.asarray(bd3).reshape(-1)[0]), np.float32)
    return W1, b1, W2, b2, W3, b3


def _host_consts(inputs):
    w3a, w3b = _conv3_lhsT(np.asarray(inputs["we3"], np.float32))
    W1, b1, W2, b2, W3, b3 = _decoder_mats(
        np.asarray(inputs["wd1"], np.float32), np.asarray(inputs["bd1"], np.float32),
        np.asarray(inputs["wd2"], np.float32), np.asarray(inputs["bd2"], np.float32),
        np.asarray(inputs["wd3"], np.float32), np.asarray(inputs["bd3"], np.float32))

    keys = np.asarray(inputs["keys"], np.float32)
    values = np.asarray(inputs["values"], np.float32)
    keys_p = np.ones((512, 128), np.float32)
    keys_p[:400] = keys
    values_p = np.zeros((512, 128), np.float32)
    values_p[:400] = values

    c = dict(_fft_consts())
    c.update({
        "w1l": _wsplit(_conv1_lhsT(np.asarray(inputs["we1"], np.float32),
                                   np.asarray(inputs["be1"], np.float32))),
        "w2l": _wsplit(_conv2_lhsT(np.asarray(inputs["we2"], np.float32))),
        "w3a": _wsplit(w3a), "w3b": _wsplit(w3b),
        "cb2": np.tile(np.asarray(inputs["be2"], np.float32), 2).reshape(128, 1),
        "cb3": np.asarray(inputs["be3"], np.float32).reshape(128, 1),
        "keys": np.ascontiguousarray(keys_p.reshape(4, 128, 128).transpose(1, 0, 2)),
        "vals": np.ascontiguousarray(values_p.reshape(4, 128, 128).transpose(1, 0, 2)),
        "ident": np.eye(128, dtype=np.float32),
        "w1d": W1,
        "w2d": np.ascontiguousarray(W2.reshape(2, 128, 4, 128).transpose(1, 0, 2, 3)),
        "w3d": np.ascontiguousarray(W3.reshape(4, 128, 16).transpose(1, 0, 2)),
        "b1d": np.ascontiguousarray(b1.reshape(2, 128).T),
        "b2d": np.ascontiguousarray(b2.reshape(4, 128).T),
        "b3row": b3.reshape(1, 16),
        "ones1": np.ones((1, 16), np.float32),
    })
    return c


def _const_specs():
    s = "stage"
    return {
        "gre": ([128, 2, NSPLIT, 256], s), "gim": ([128, 2, NSPLIT, 256], s),
        "gimn": ([128, 2, NSPLIT, 256], s),
        "w1l": ([128, 1, NSPLIT, 128], s), "w2l": ([128, 3, NSPLIT, 128], s),
        "w3a": ([128, 3, NSPLIT, 128], s), "w3b": ([64, 3, NSPLIT, 128], s),
        "cb2": ([128, 1], "f32"), "cb3": ([128, 1], "f32"),
        "keys": ([128, 4, 128], "f32"), "vals": ([128, 4, 128], "f32"),
        "ident": ([128, 128], "f32"),
        "w1d": ([128, 256], "f32"), "w2d": ([128, 2, 4, 128], "f32"),
        "w3d": ([128, 4, 16], "f32"),
        "b1d": ([128, 2], "f32"), "b2d": ([128, 4], "f32"),
        "b3row": ([1, 16], "f32"), "ones1": ([1, 16], "f32"),
    }


def mk(t, poff, pstep, pcount, fdims, foff=0):
    """Manual AP on tile t (element units; partition pitch from the tile AP)."""
    pitch = t.ap[0][0]
    dims = [[pstep * pitch, pcount]] + [list(d) for d in fdims]
    return bass.AP(t.tensor, t.offset + poff * pitch + foff, dims)


def dramap(t, off, dims):
    return bass.AP(t.tensor, t.offset + off, [list(d) for d in dims])


# ---------------------------------------------------------------------------
# kernel builder
# ---------------------------------------------------------------------------
def build_nc(b_loc=16):
    sdt = _sdt()
    fft_in_dt = BF16 if STAGE == "bf16" else F32
    nc = bacc.Bacc("TRN2", target_bir_lowering=False, debug=False)

    x_in = nc.dram_tensor("x_in", [b_loc, 128, 2, 256], fft_in_dt,
                          kind="ExternalInput")
    out_d = nc.dram_tensor("out", [b_loc, 16], F32, kind="ExternalOutput")
    qdbg_d = (nc.dram_tensor("qdbg", [128, b_loc], F32, kind="ExternalOutput")
              if os.environ.get("K_DBGQ") else None)
    hdbg_d = None
    if os.environ.get("K_DBGH"):
        _ddt = _sdt()
        hdbg_d = {
            "dxm": nc.dram_tensor("dxm", [128, 2, 258], _ddt,
                                  kind="ExternalOutput"),
            "dh1": nc.dram_tensor("dh1", [64, 64, 130], _ddt,
                                  kind="ExternalOutput"),
            "dh2": nc.dram_tensor("dh2", [64, 64, 66], _ddt,
                                  kind="ExternalOutput"),
            "ds2": nc.dram_tensor("ds2", [128, 64, 130], _ddt,
                                  kind="ExternalOutput"),
        }
    const_d = {}
    for name, (shape, kind) in _const_specs().items():
        dt_ = _sdt() if kind == "stage" else F32
        const_d[name] = nc.dram_tensor(name, shape, dt_, kind="ExternalInput")

    with tile.TileContext(nc) as tc:
        from contextlib import ExitStack
        with ExitStack() as ctx:
            cpool = ctx.enter_context(tc.tile_pool(name="consts", bufs=1))
            spool = ctx.enter_context(tc.tile_pool(name="stage", bufs=1))
            xpool = ctx.enter_context(tc.tile_pool(name="xin", bufs=3))
            wpool = ctx.enter_context(tc.tile_pool(name="work", bufs=2))
            rpool = ctx.enter_context(tc.tile_pool(name="ret", bufs=1))
            dpool = ctx.enter_context(tc.tile_pool(name="dram", bufs=1, space="DRAM"))
            fftps = ctx.enter_context(tc.tile_pool(name="fftps", bufs=3, space="PSUM"))
            convps = ctx.enter_context(tc.tile_pool(name="convps", bufs=3, space="PSUM"))
            miscps = ctx.enter_context(tc.tile_pool(name="miscps", bufs=2, space="PSUM"))

            # critical-path consts (FFT G matrices + conv1 weights) load
            # first on the sync queue; everything else goes on the gpsimd
            # queue so image-0's FFT isn't stuck behind ~1MB of decoder
            # weights.
            crit = ("gre", "gim", "gimn", "w1l")
            cs = {}
            for name, (shape, kind) in _const_specs().items():
                dt_ = _sdt() if kind == "stage" else F32
                t = cpool.tile(shape, dt_, name=f"c_{name}")
                if name in crit:
                    nc.sync.dma_start(out=t, in_=const_d[name].ap())
                cs[name] = t

            # fixed stage buffers (all single-buffered; WAR deps order reuse)
            strip1 = spool.tile([128, 2, 32, 256], sdt, name="strip1")
            strip2 = spool.tile([128, 64, 130], sdt, name="strip2")
            strip3a = spool.tile([128, 64, 66], sdt, name="strip3a")
            strip3b = spool.tile([64, 64, 66], sdt, name="strip3b")
            xpooled1 = spool.tile([128, 64, 130], sdt, name="xpooled1")
            xpB1 = spool.tile([64, 64, 130], sdt, name="xpB1")
            h1X = spool.tile([64, 64, 130], sdt, name="h1X")
            xpooled2 = spool.tile([128, 64, 66], sdt, name="xpooled2")
            xpB2 = spool.tile([64, 64, 66], sdt, name="xpB2")
            h2buf = spool.tile([64, 64, 66], sdt, name="h2buf")
            xm = spool.tile([128, 2, 258], sdt, name="xm")
            qacc = spool.tile([128, 8], F32, name="qacc")
            qT = spool.tile([128, b_loc], F32, name="qT")
            xmd = dpool.tile([258, 258], sdt, name="xmd")

            for t in (strip1, strip2, strip3a, strip3b, xpooled1, xpooled2):
                nc.vector.memset(t, 0.0)
            nc.vector.memset(xm, 0.0)
            nc.vector.memset(strip1[0:1], 1.0)  # conv1 bias row (ones)
            zrow = cpool.tile([1, 2, 258], sdt, name="zrow")
            nc.vector.memset(zrow, 0.0)
            nc.sync.dma_start(  # xmd pad rows 0, 257 (cols padded per-write)
                out=dramap(xmd, 0, [[1, 1], [257 * 258, 2], [1, 258]]),
                in_=zrow)

            # ---------------- per-image pipeline helpers ----------------
            def load_x(img):
                t = xpool.tile([128, 2, 256], fft_in_dt, name="x_sb",
                               tag="x_sb")
                nc.gpsimd.dma_start(
                    out=t,
                    in_=dramap(x_in.ap(), img * 65536,
                               [[512, 128], [256, 2], [1, 256]]))
                return t

            def fft_step1(x_sb):
                """step1: yts[(nm, mt)] sbuf bf16 tiles [128(x), 256(u)]."""
                yts = {}
                for mt in range(2):
                    for nm, rt in (("re", "gre"), ("im", "gim")):
                        ps = fftps.tile([128, 256], F32, name="ps_yt",
                                        tag="fft")
                        n_mm = 2 * NSPLIT
                        i = 0
                        for kt in range(2):
                            for sp in range(NSPLIT):
                                nc.tensor.matmul(
                                    ps,
                                    x_sb[:, kt, mt * 128:(mt + 1) * 128],
                                    cs[rt][:, kt, sp, :],
                                    start=(i == 0), stop=(i == n_mm - 1))
                                i += 1
                        sb = wpool.tile([128, 256], fft_in_dt,
                                        name=f"yt{nm}{mt}", tag=f"yt{nm}{mt}")
                        nc.scalar.copy(sb, ps)
                        yts[(nm, mt)] = sb
                return yts

            def fft_step2_mag(yts):
                """step2 + magnitude -> xm [128, 2, 258] (padded cols)."""
                for mt in range(2):
                    zre = fftps.tile([128, 256], F32, name="ps_zre", tag="fft")
                    zim = fftps.tile([128, 256], F32, name="ps_zim", tag="fft")
                    for out_ps, combos in (
                        (zre, [("re", "gre"), ("im", "gimn")]),
                        (zim, [("re", "gim"), ("im", "gre")]),
                    ):
                        n_mm = 4 * NSPLIT
                        i = 0
                        for nm, rt in combos:
                            for kt in range(2):
                                for sp in range(NSPLIT):
                                    nc.tensor.matmul(
                                        out_ps,
                                        yts[(nm, kt)][:, mt * 128:(mt + 1) * 128],
                                        cs[rt][:, kt, sp, :],
                                        start=(i == 0), stop=(i == n_mm - 1))
                                    i += 1
                    t1 = wpool.tile([128, 256], F32, name="mag1", tag="mag1")
                    t2 = wpool.tile([128, 256], F32, name="mag2", tag="mag2")
                    nc.scalar.square(t1, zre)
                    nc.scalar.square(t2, zim)
                    nc.vector.tensor_add(t1, t1, t2)
                    nc.scalar.sqrt(mk(xm, 0, 1, 128, [[1, 256]], mt * 258 + 1),
                                   t1)

            dma_q = nc.sync if os.environ.get("K_SYNCQ") else nc.gpsimd

            def xm_to_dram_and_strips(hs_list=(0, 1)):
                # full 258-wide rows (pads included) -> contiguous-ish writes
                dma_q.dma_start(
                    out=dramap(xmd, 258,
                               [[258, 128], [128 * 258, 2], [1, 258]]),
                    in_=xm)
                for hs in hs_list:
                    for dx in range(3):
                        dma_q.dma_start(
                            out=mk(strip1, 1 + 6 * dx, 1, 6,
                                   [[256, 32], [1, 256]], hs * 8192),
                            in_=dramap(xmd, 33024 * hs + dx,
                                       [[258, 6], [1032, 32], [1, 256]]))

            def conv1_half(hs):
                for ch in range(16):
                    sg = 32 * hs + 2 * ch
                    ps = convps.tile([128, 512], F32, name="c1ps", tag="conv")
                    for sp in range(NSPLIT):
                        nc.tensor.matmul(
                            ps, cs["w1l"][:, 0, sp, :],
                            strip1[:, hs, 2 * ch:2 * ch + 2, :],
                            start=(sp == 0), stop=(sp == NSPLIT - 1))
                    # relu on ACT (bias is in K-row 18), x-pool on DVE
                    rt = wpool.tile([128, 2, 256], sdt, name="rt1", tag="rt1",
                                    bufs=3)
                    nc.scalar.activation(rt, ps, ActFn.Relu)
                    nc.vector.tensor_max(
                        mk(xpooled1, 0, 1, 128, [[130, 2], [1, 128]],
                           sg * 130 + 1),
                        mk(rt, 0, 1, 128, [[256, 2], [2, 128]], 0),
                        mk(rt, 0, 1, 128, [[256, 2], [2, 128]], 1))
                # per-half y-pool: align upper partitions, fold into h1X
                lo, n = hs * 32 * 130, 32 * 130
                nc.sync.dma_start(
                    out=mk(xpB1, 0, 1, 64, [[1, n]], lo),
                    in_=mk(xpooled1, 64, 1, 64, [[1, n]], lo))
                nc.vector.tensor_max(
                    mk(h1X, 0, 1, 64, [[1, n]], lo),
                    mk(xpooled1, 0, 1, 64, [[1, n]], lo),
                    mk(xpB1, 0, 1, 64, [[1, n]], lo))

            def fills2():
                # strip2 fills: slot s2 of j holds h1 row 2*s2+j-1;
                # h1X partitions 0..31 = even rows (slot=y/2), 32..63 = odd.
                for j, d0, ns, g, s0 in ((0, 1, 63, 32, 0), (1, 0, 64, 0, 0),
                                         (2, 0, 64, 32, 0), (3, 0, 63, 0, 1)):
                    nc.sync.dma_start(
                        out=mk(strip2, 32 * j, 1, 32, [[1, ns * 130]],
                               d0 * 130),
                        in_=mk(h1X, g, 1, 32, [[1, ns * 130]], s0 * 130))

            def conv2_half(half):
                for ch in range(8 * half, 8 * half + 8):
                    ps = convps.tile([128, 512], F32, name="c2ps", tag="conv")
                    i = 0
                    for dx in range(3):
                        for sp in range(NSPLIT):
                            nc.tensor.matmul(
                                ps, cs["w2l"][:, dx, sp, :],
                                mk(strip2, 0, 1, 128, [[130, 4], [1, 128]],
                                   4 * ch * 130 + dx),
                                start=(i == 0), stop=(i == 3 * NSPLIT - 1))
                            i += 1
                    rt2 = wpool.tile([128, 4, 128], sdt, name="rt2", tag="rt2",
                                     bufs=3)
                    nc.scalar.activation(rt2, ps, ActFn.Relu,
                                         bias=cs["cb2"][:, 0:1])
                    nc.vector.tensor_max(
                        mk(xpooled2, 0, 1, 128, [[66, 4], [1, 64]],
                           4 * ch * 66 + 1),
                        mk(rt2, 0, 1, 128, [[128, 4], [2, 64]], 0),
                        mk(rt2, 0, 1, 128, [[128, 4], [2, 64]], 1))
                # per-half align + fold into h2buf (strip3 fills happen
                # later, after the previous image's conv3 has consumed the
                # strips)
                lo, n = half * 32 * 66, 32 * 66
                nc.sync.dma_start(
                    out=mk(xpB2, 0, 1, 64, [[1, n]], lo),
                    in_=mk(xpooled2, 64, 1, 64, [[1, n]], lo))
                nc.vector.tensor_max(
                    mk(h2buf, 0, 1, 64, [[1, n]], lo),
                    mk(xpooled2, 0, 1, 64, [[1, n]], lo),
                    mk(xpB2, 0, 1, 64, [[1, n]], lo))

            def fills3():
                # strip3a: d=0 partitions hold row s-1, d=1 hold row s;
                # strip3b holds row s+1 (edge slots stay zero from init).
                for st, p0, d0, ns, s0 in (
                        (strip3a, 0, 1, 63, 0), (strip3a, 64, 0, 64, 0),
                        (strip3b, 0, 0, 63, 1)):
                    nc.sync.dma_start(
                        out=mk(st, p0, 1, 64, [[1, ns * 66]], d0 * 66),
                        in_=mk(h2buf, 0, 1, 64, [[1, ns * 66]], s0 * 66))

            def conv3_all(img):
                for ch in range(8):
                    ps = convps.tile([128, 512], F32, name="c3ps", tag="conv")
                    n_mm = 6 * NSPLIT
                    i = 0
                    for dx in range(3):
                        for w_, st3, pc in (("w3a", strip3a, 128),
                                            ("w3b", strip3b, 64)):
                            for sp in range(NSPLIT):
                                nc.tensor.matmul(
                                    ps, cs[w_][:, dx, sp, :],
                                    mk(st3, 0, 1, pc, [[66, 8], [1, 64]],
                                       8 * ch * 66 + dx),
                                    start=(i == 0), stop=(i == n_mm - 1))
                                i += 1
                    scr = wpool.tile([128, 512], F32, name="scr3", tag="scr3",
                                     bufs=2)
                    nc.scalar.activation(scr, ps, ActFn.Relu,
                                         bias=cs["cb3"][:, 0:1],
                                         accum_out=qacc[:, ch:ch + 1])
                nc.vector.reduce_sum(qT[:, img:img + 1], qacc, axis=AxX)

            # ---------------- software-pipelined image loop ----------------
            rep = int(os.environ.get("K_REP", "1"))
            loop_cm = tc.For_i(0, rep, 1) if rep > 1 else None
            if loop_cm is not None:
                loop_cm.__enter__()

            x_tiles = {0: load_x(0)}
            if b_loc > 1:
                x_tiles[1] = load_x(1)

            def load_misc_consts():
                for name, (shape, kind) in _const_specs().items():
                    if name not in crit:
                        nc.gpsimd.dma_start(out=cs[name],
                                            in_=const_d[name].ap())
            if os.environ.get("K_NOSKEW"):
                for i in range(b_loc):
                    yts = fft_step1(x_tiles[i])
                    fft_step2_mag(yts)
                    xm_to_dram_and_strips()
                    if i == 0:
                        load_misc_consts()
                    if i + 2 < b_loc:
                        x_tiles[i + 2] = load_x(i + 2)
                    conv1_half(0)
                    conv1_half(1)
                    fills2()
                    conv2_half(0)
                    conv2_half(1)
                    fills3()
                    conv3_all(i)
                    x_tiles.pop(i, None)
            else:
                # 3-deep skew: iteration i runs FFT(i+1), conv1(i),
                # conv2(i-1) and conv3(i-2), so every staging chain has at
                # least a full conv phase of matmuls to hide behind.
                yts = fft_step1(x_tiles[0])
                fft_step2_mag(yts)
                xm_to_dram_and_strips()
                load_misc_consts()
                for i in range(b_loc):
                    if i + 1 < b_loc:
                        yts = fft_step1(x_tiles[i + 1])
                    conv1_half(0)
                    conv1_half(1)
                    if i + 1 < b_loc:
                        fft_step2_mag(yts)
                        xm_to_dram_and_strips()
                    if i + 2 < b_loc:
                        x_tiles[i + 2] = load_x(i + 2)
                    if i >= 1:
                        conv2_half(0)
                        conv2_half(1)
                    fills2()
                    if i >= 2:
                        conv3_all(i - 2)
                    if i >= 1:
                        fills3()
                    x_tiles.pop(i, None)
                conv2_half(0)
                conv2_half(1)
                if b_loc >= 2:
                    conv3_all(b_loc - 2)
                fills3()
                conv3_all(b_loc - 1)

            # ---------------- retrieval (fp32) ----------------
            # key normalization -> knT [128, 400] (fp32)
            knT = rpool.tile([128, 400], F32, name="knT")
            ksq = rpool.tile([128, 4, 128], F32, name="ksq")
            nc.vector.tensor_mul(ksq, cs["keys"], cs["keys"])
            kss = rpool.tile([128, 4], F32, name="kss")
            nc.vector.reduce_sum(kss, ksq, axis=AxX)
            knm = rpool.tile([128, 4], F32, name="knm")
            nc.scalar.sqrt(knm, kss)
            nc.vector.tensor_scalar_max(knm, knm, 1e-12)
            kri = rpool.tile([128, 4], F32, name="kri")
            nc.vector.reciprocal(kri, knm)
            knrm = rpool.tile([128, 4, 128], F32, name="knrm")
            for c in range(4):
                nc.vector.tensor_scalar_mul(
                    knrm[:, c, :], cs["keys"][:, c, :], kri[:, c:c + 1])
            for c in range(4):
                pc = 128 if c < 3 else 16
                tp = miscps.tile([128, 128], F32, name="tp_kn", tag="misc")
                nc.tensor.transpose(
                    tp[:, :pc], knrm[:pc, c, :], cs["ident"][:pc, :pc])
                nc.scalar.copy(knT[:, c * 128:c * 128 + pc], tp[:, :pc])

            if qdbg_d is not None:
                nc.sync.dma_start(out=qdbg_d.ap(), in_=qT)
            if hdbg_d is not None:
                # dump last-image staging tiles raw (bf16)
                for nm_, src_ in (("dxm", xm), ("dh1", h1X), ("dh2", h2buf),
                                  ("ds2", strip2)):
                    nc.sync.dma_start(out=hdbg_d[nm_].ap(), in_=src_)
            bl = b_loc
            simps = miscps.tile([bl, 400], F32, name="simps", tag="misc")
            nc.tensor.matmul(simps, qT, knT, start=True, stop=True)
            gram = miscps.tile([bl, bl], F32, name="gram", tag="misc")
            nc.tensor.matmul(gram, qT, qT, start=True, stop=True)
            gd = rpool.tile([bl, bl], F32, name="gd")
            nc.vector.tensor_mul(gd, gram, cs["ident"][:bl, :bl])
            q2 = rpool.tile([bl, 1], F32, name="q2")
            nc.vector.reduce_sum(q2, gd, axis=AxX)
            qn = rpool.tile([bl, 1], F32, name="qn")
            nc.scalar.sqrt(qn, q2)
            nc.vector.tensor_scalar_max(qn, qn, 1e-12)
            rq = rpool.tile([bl, 1], F32, name="rq")
            nc.vector.reciprocal(rq, qn)
            sim = rpool.tile([bl, 400], F32, name="sim")
            nc.vector.tensor_scalar_mul(sim, simps, rq[:, 0:1])

            cur = rpool.tile([bl, 400], F32, name="cur")
            nc.vector.tensor_copy(cur, sim)
            m1 = rpool.tile([bl, 1], F32, name="m1")
            nc.vector.reduce_max(m1, sim, axis=AxX)
            msk = rpool.tile([bl, 400], F32, name="msk")
            mk_ = m1
            for it in range(4):
                nc.vector.tensor_scalar(msk, cur, mk_[:, 0:1], None,
                                        op0=AluOp.is_ge)
                nc.vector.scalar_tensor_tensor(cur, msk, -1e30, cur,
                                               op0=AluOp.mult, op1=AluOp.add)
                nm_ = rpool.tile([bl, 1], F32, name=f"mk{it}")
                nc.vector.reduce_max(nm_, cur, axis=AxX)
                mk_ = nm_
            m5 = mk_
            nc.vector.tensor_scalar(msk, sim, m5[:, 0:1], None, op0=AluOp.is_ge)
            m1n = rpool.tile([bl, 1], F32, name="m1n")
            nc.vector.tensor_scalar_mul(m1n, m1, -1.0)
            es = rpool.tile([bl, 400], F32, name="es")
            nc.scalar.activation(es, sim, ActFn.Exp, bias=m1n[:, 0:1])
            ew = rpool.tile([bl, 400], F32, name="ew")
            nc.vector.tensor_mul(ew, es, msk)
            zs = rpool.tile([bl, 1], F32, name="zs")
            nc.vector.reduce_sum(zs, ew, axis=AxX)
            rz = rpool.tile([bl, 1], F32, name="rz")
            nc.vector.reciprocal(rz, zs)
            nc.vector.tensor_scalar_mul(ew, ew, rz[:, 0:1])

            eT = rpool.tile([128, 4, bl], F32, name="eT")
            for c in range(4):
                pc = 128 if c < 3 else 16
                tp = miscps.tile([128, bl], F32, name="tp_e", tag="misc")
                nc.tensor.transpose(tp[:pc, :], ew[:, c * 128:c * 128 + pc],
                                    cs["ident"][:bl, :bl])
                nc.scalar.copy(eT[:pc, c, :], tp[:pc, :])

            memps = miscps.tile([128, bl], F32, name="memps", tag="misc")
            for c in range(4):
                pc = 128 if c < 3 else 16
                nc.tensor.matmul(memps, cs["vals"][:pc, c, :], eT[:pc, c, :],
                                 start=(c == 0), stop=(c == 3))
            memT = rpool.tile([128, bl], F32, name="memT")
            nc.scalar.copy(memT, memps)

            h1T = rpool.tile([128, 2, bl], F32, name="h1T")
            for mt in range(2):
                ps = miscps.tile([128, bl], F32, name="d1ps", tag="misc")
                nc.tensor.matmul(ps, cs["w1d"][:, mt * 128:(mt + 1) * 128],
                                 memT, start=True, stop=True)
                nc.scalar.activation(h1T[:, mt, :], ps, ActFn.Relu,
                                     bias=cs["b1d"][:, mt:mt + 1])
            h2T = rpool.tile([128, 4, bl], F32, name="h2T")
            for mt in range(4):
                ps = miscps.tile([128, bl], F32, name="d2ps", tag="misc")
                for kt in range(2):
                    nc.tensor.matmul(ps, cs["w2d"][:, kt, mt, :], h1T[:, kt, :],
                                     start=(kt == 0), stop=(kt == 1))
                nc.scalar.activation(h2T[:, mt, :], ps, ActFn.Relu,
                                     bias=cs["b2d"][:, mt:mt + 1])
            ops = miscps.tile([bl, 16], F32, name="outps", tag="misc")
            for c in range(4):
                nc.tensor.matmul(ops, h2T[:, c, :], cs["w3d"][:, c, :],
                                 start=(c == 0), stop=False)
            nc.tensor.matmul(ops, cs["ones1"][:, :bl], cs["b3row"],
                             start=False, stop=True)
            out_sb = rpool.tile([bl, 16], F32, name="out_sb")
            nc.scalar.copy(out_sb, ops)
            nc.sync.dma_start(out=out_d.ap(), in_=out_sb)
            if loop_cm is not None:
                loop_cm.__exit__(None, None, None)

    nc.compile()
    return nc


# ---------------------------------------------------------------------------
# host entry
# ---------------------------------------------------------------------------
_NC_CACHE = {}


def _get_nc(b_loc):
    key = (b_loc, STAGE, os.environ.get("K_REP", "1"),
           os.environ.get("K_NOSKEW"), os.environ.get("K_SYNCQ"),
           os.environ.get("K_DBGQ"), os.environ.get("K_DBGH"))
    if key not in _NC_CACHE:
        _NC_CACHE[key] = build_nc(b_loc)
    return _NC_CACHE[key]


def _pack_x(x_shard):
    b = x_shard.shape[0]
    xr = np.ascontiguousarray(
        x_shard.reshape(b, 2, 128, 256).transpose(0, 2, 1, 3)).astype(np.float32)
    return xr.astype(_np_sdt())


def kernel(**inputs):
    x = np.asarray(inputs["x"], np.float32)
    # jnp.fft.fftshift also shifts the batch axis: output b uses x[(b+64)%128]
    xp = np.roll(x, -64, axis=0)
    consts = _host_consts(inputs)

    b_loc = B // N_CORES
    nc = _get_nc(b_loc)

    in_maps = []
    for c in range(N_CORES):
        m = dict(consts)
        m["x_in"] = _pack_x(xp[c * b_loc:(c + 1) * b_loc])
        in_maps.append(m)

    kwargs = {}
    if os.environ.get("K_TRACE"):
        kwargs["trace"] = True
    res = run_bass_kernel_spmd(nc, in_maps, core_ids=list(range(N_CORES)),
                               **kwargs)
    global LAST_RESULTS
    LAST_RESULTS = res
    out = np.concatenate([r["out"] for r in res.results], axis=0)
    return out.reshape(B, 1, 4, 4).astype(np.float32)


LAST_RESULTS = None


if __name__ == "__main__":
    build_nc(int(os.environ.get("K_BLOC", "1")))
    print("built ok")


# revision 42
# speedup vs baseline: 1.3468x; 1.0413x over previous
"""Trainium2 Bass kernel for nn_FFTMemAutoEncoderBranch (retrieval_knn).

Data-parallel over batch: 8 cores x 16 images, no cross-core communication.

Numerics: the problem's top-5 retrieval runs on near-identical queries (white
-noise FFT magnitudes), with 5th/6th similarity gaps down to 7e-6 -- so conv
weights and DFT matrices must act at ~fp32 fidelity while activations tolerate
bf16. Scheme (validated against the reference on HW, 0/128 top-5 flips):
  - activations/staging in bf16
  - every stationary operand (DFT matrices G, conv weights) is split
    W = hi + lo into two bf16 matmuls accumulating in fp32 PSUM
  - retrieval + decoder in fp32

Performance structure (v2): the v1 kernel ran image-major with serial staging,
leaving the PE idle ~80us/image (trace: 1.25ms of gaps in a 3.2ms span, HAM
re-throttling the PE to 1.2GHz for ~85% of the run) and moving ~100MB/core of
SBUF<->SBUF staging in 256-512B DMA descriptors. v2:
  - software-pipelines images with a one-iteration skew: PE order is
    [FFTs1(i+1) | conv1(i) | FFTs2(i+1) | conv2(i) | conv3(i)], so every
    staging chain (mag->DRAM bounce->strip1; pool->align->fold->strip fills)
    runs in the shadow of ~30-90us of matmuls from the neighboring image.
  - staging tiles use pitch-matched padded rows (130-wide for conv2 strips,
    66-wide for conv3) so each strip fill is a handful of multi-KB-contiguous
    descriptors instead of thousands of 256B ones.
  - conv1 folds bias into a 19th K-row (rhs row of ones) and fuses
    relu+x-pool into one DVE scalar_tensor_tensor reading PSUM directly.
  - pool/align/fold/fill chains run per half-image so conv(i) chunk k's rhs
    is staged while chunks k-1 of the same image still run.

Per image: FFT2 as DFT matmuls (z = G x G^T, G = roll(F,128,0)/16, fftshift+
ortho folded in; batch roll done on host), conv1 strips via a DRAM bounce of
the padded 258x258 magnitude image, conv2/conv3 strips via parity-split SBUF
copies, maxpools on DVE, conv3 relu accumulating into q via ACT accum_out.
Retrieval: fp32 sim, top-5 threshold via 5x(reduce_max+mask), masked stable
softmax, mem = values^T @ e^T; decoder collapsed to 3 dense matmuls.
"""

import os
import sys
import numpy as np

for _p in ("/opt/trn_rl_repo", "/root/.axon_site/_ro/trn_rl_repo"):
    if os.path.isdir(_p) and _p not in sys.path:
        sys.path.append(_p)

import concourse.bass as bass
import concourse.mybir as mybir
import concourse.tile as tile
from concourse import bacc
from concourse.bass_utils import run_bass_kernel_spmd

F32 = mybir.dt.float32
BF16 = mybir.dt.bfloat16

N_CORES = 8
B = 128
H = 256

# STAGE: "bf16" (hi/lo-split weights, bf16 activations) | "f32" (all fp32)
STAGE = os.environ.get("K_STAGE", "bf16")
NSPLIT = 2 if STAGE == "bf16" else 1

AluOp = mybir.AluOpType
ActFn = mybir.ActivationFunctionType
AxX = mybir.AxisListType.X


def _sdt():
    return BF16 if STAGE == "bf16" else F32


def _np_sdt():
    if STAGE == "bf16":
        import ml_dtypes
        return ml_dtypes.bfloat16
    return np.float32


# ---------------------------------------------------------------------------
# host-side constant construction
# ---------------------------------------------------------------------------
def _pack2(m):  # [256, N] -> [128, 2, N]
    return np.ascontiguousarray(m.reshape(2, 128, -1).transpose(1, 0, 2))


def _fft_consts():
    k = np.arange(H)
    F = np.exp(-2j * np.pi * np.outer(k, k) / H) / 16.0
    G = np.roll(F, H // 2, axis=0)
    GT = G.T.copy()
    out = {}
    for name, m in (("gre", GT.real), ("gim", GT.imag), ("gimn", -GT.imag)):
        m = m.astype(np.float32)
        if NSPLIT == 1:
            out[name] = _pack2(m)[:, :, None, :]  # [128, 2, 1, 256] f32
        else:
            sdt = _np_sdt()
            hi32 = m.astype(sdt).astype(np.float32)
            lo = (m - hi32).astype(sdt)
            hi = m.astype(sdt)
            # [128, 2, 2, 256]: (part-of-256-rows, half, split, col)
            out[name] = np.stack([_pack2(hi), _pack2(lo)], axis=2)
    return out


T1ORD = (0, 2, 1, 3)  # conv1 M block -> strip row offset t; so that y-pool
# pairs (t0,t1),(t2,t3) become max(partitions 0:64, partitions 64:128)


def _conv1_lhsT(we1, be1):
    # K = (dx, j) packed on partitions 1..18, p = 1 + dx*6 + j (dx-major so
    # each strip1 fill DMA writes a contiguous partition block); partition 0
    # is the bias row (strip1 partition 0 holds ones; engine-op partition
    # bases must be 32-aligned, so the ones memset needs base 0).
    out = np.zeros((128, 1, 128), np.float32)
    for dx in range(3):
        for m in range(128):
            t, co = T1ORD[m // 32], m % 32
            for j in range(6):
                if 0 <= j - t <= 2:
                    out[1 + dx * 6 + j, 0, m] = we1[co, 0, j - t, dx]
    for m in range(128):
        out[0, 0, m] = be1[m % 32]
    return out


def _conv2_lhsT(we2):
    # K layout j-major: k = j*32 + ci (each strip2 fill writes a contiguous
    # partition block)
    out = np.zeros((128, 3, 128), np.float32)
    for dx in range(3):
        for m in range(128):
            t, co = m // 64, m % 64
            for k in range(128):
                ci, j = k % 32, k // 32
                if 0 <= j - t <= 2:
                    out[k, dx, m] = we2[co, ci, j - t, dx]
    return out


def _conv3_lhsT(we3):
    # K layout d-major: k = d*64 + ci
    A = np.zeros((128, 3, 128), np.float32)
    Bm = np.zeros((64, 3, 128), np.float32)
    for dx in range(3):
        for k in range(128):
            ci, d = k % 64, k // 64
            A[k, dx, :] = we3[:, ci, d, dx]
        for ci in range(64):
            Bm[ci, dx, :] = we3[:, ci, 2, dx]
    return A, Bm


def _wsplit(w):
    """[P, D, N] -> [P, D, NSPLIT, N] staging dtype (hi, lo)."""
    sdt = _np_sdt()
    if NSPLIT == 1:
        return w[:, :, None, :].astype(np.float32)
    hi32 = w.astype(sdt).astype(np.float32)
    lo = (w - hi32).astype(sdt)
    return np.stack([w.astype(sdt), lo], axis=2)


def _decoder_mats(wd1, bd1, wd2, bd2, wd3, bd3):
    W1 = np.zeros((128, 256), np.float32)
    for c in range(64):
        for i in range(2):
            for j in range(2):
                W1[:, c * 4 + i * 2 + j] = wd1[:, c, i + 1, j + 1]
    b1 = np.repeat(bd1, 4).astype(np.float32)

    W2 = np.zeros((256, 512), np.float32)
    for c in range(64):
        for ii in range(2):
            for jj in range(2):
                f = c * 4 + ii * 2 + jj
                for c2 in range(32):
                    for y in range(4):
                        ky = y + 1 - 2 * ii
                        if not (0 <= ky <= 3):
                            continue
                        for x in range(4):
                            kx = x + 1 - 2 * jj
                            if 0 <= kx <= 3:
                                W2[f, c2 * 16 + y * 4 + x] = wd2[c, c2, ky, kx]
    b2 = np.repeat(bd2, 16).astype(np.float32)

    W3 = np.zeros((512, 16), np.float32)
    for c2 in range(32):
        for y in range(4):
            for x in range(4):
                g = c2 * 16 + y * 4 + x
                for oy in range(4):
                    ky = y - oy + 1
                    if not (0 <= ky <= 2):
                        continue
                    for ox in range(4):
                        kx = x - ox + 1
                        if 0 <= kx <= 2:
                            W3[g, oy * 4 + ox] = wd3[0, c2, ky, kx]
    b3 = np.full((16,), float(np.asarray(bd3).reshape(-1)[0]), np.float32)
    return W1, b1, W2, b2, W3, b3


def _host_consts(inputs):
    w3a, w3b = _conv3_lhsT(np.asarray(inputs["we3"], np.float32))
    W1, b1, W2, b2, W3, b3 = _decoder_mats(
        np.asarray(inputs["wd1"], np.float32), np.asarray(inputs["bd1"], np.float32),
        np.asarray(inputs["wd2"], np.float32), np.asarray(inputs["bd2"], np.float32),
        np.asarray(inputs["wd3"], np.float32), np.asarray(inputs["bd3"], np.float32))

    keys = np.asarray(inputs["keys"], np.float32)
    values = np.asarray(inputs["values"], np.float32)
    keys_p = np.ones((512, 128), np.float32)
    keys_p[:400] = keys
    values_p = np.zeros((512, 128), np.float32)
    values_p[:400] = values

    c = dict(_fft_consts())
    c.update({
        "w1l": _wsplit(_conv1_lhsT(np.asarray(inputs["we1"], np.float32),
                                   np.asarray(inputs["be1"], np.float32))),
        "w2l": _wsplit(_conv2_lhsT(np.asarray(inputs["we2"], np.float32))),
        "w3a": _wsplit(w3a), "w3b": _wsplit(w3b),
        "cb2": np.tile(np.asarray(inputs["be2"], np.float32), 2).reshape(128, 1),
        "cb3": np.asarray(inputs["be3"], np.float32).reshape(128, 1),
        "keys": np.ascontiguousarray(keys_p.reshape(4, 128, 128).transpose(1, 0, 2)),
        "vals": np.ascontiguousarray(values_p.reshape(4, 128, 128).transpose(1, 0, 2)),
        "ident": np.eye(128, dtype=np.float32),
        "w1d": W1,
        "w2d": np.ascontiguousarray(W2.reshape(2, 128, 4, 128).transpose(1, 0, 2, 3)),
        "w3d": np.ascontiguousarray(W3.reshape(4, 128, 16).transpose(1, 0, 2)),
        "b1d": np.ascontiguousarray(b1.reshape(2, 128).T),
        "b2d": np.ascontiguousarray(b2.reshape(4, 128).T),
        "b3row": b3.reshape(1, 16),
        "ones1": np.ones((1, 16), np.float32),
    })
    return c


def _const_specs():
    s = "stage"
    return {
        "gre": ([128, 2, NSPLIT, 256], s), "gim": ([128, 2, NSPLIT, 256], s),
        "gimn": ([128, 2, NSPLIT, 256], s),
        "w1l": ([128, 1, NSPLIT, 128], s), "w2l": ([128, 3, NSPLIT, 128], s),
        "w3a": ([128, 3, NSPLIT, 128], s), "w3b": ([64, 3, NSPLIT, 128], s),
        "cb2": ([128, 1], "f32"), "cb3": ([128, 1], "f32"),
        "keys": ([128, 4, 128], "f32"), "vals": ([128, 4, 128], "f32"),
        "ident": ([128, 128], "f32"),
        "w1d": ([128, 256], "f32"), "w2d": ([128, 2, 4, 128], "f32"),
        "w3d": ([128, 4, 16], "f32"),
        "b1d": ([128, 2], "f32"), "b2d": ([128, 4], "f32"),
        "b3row": ([1, 16], "f32"), "ones1": ([1, 16], "f32"),
    }


def mk(t, poff, pstep, pcount, fdims, foff=0):
    """Manual AP on tile t (element units; partition pitch from the tile AP)."""
    pitch = t.ap[0][0]
    dims = [[pstep * pitch, pcount]] + [list(d) for d in fdims]
    return bass.AP(t.tensor, t.offset + poff * pitch + foff, dims)


def dramap(t, off, dims):
    return bass.AP(t.tensor, t.offset + off, [list(d) for d in dims])


# ---------------------------------------------------------------------------
# kernel builder
# ---------------------------------------------------------------------------
def build_nc(b_loc=16):
    sdt = _sdt()
    fft_in_dt = BF16 if STAGE == "bf16" else F32
    nc = bacc.Bacc("TRN2", target_bir_lowering=False, debug=False)

    x_in = nc.dram_tensor("x_in", [b_loc, 128, 2, 256], fft_in_dt,
                          kind="ExternalInput")
    out_d = nc.dram_tensor("out", [b_loc, 16], F32, kind="ExternalOutput")
    qdbg_d = (nc.dram_tensor("qdbg", [128, b_loc], F32, kind="ExternalOutput")
              if os.environ.get("K_DBGQ") else None)
    hdbg_d = None
    if os.environ.get("K_DBGH"):
        _ddt = _sdt()
        hdbg_d = {
            "dxm": nc.dram_tensor("dxm", [128, 2, 258], _ddt,
                                  kind="ExternalOutput"),
            "dh1": nc.dram_tensor("dh1", [64, 64, 130], _ddt,
                                  kind="ExternalOutput"),
            "dh2": nc.dram_tensor("dh2", [64, 64, 66], _ddt,
                                  kind="ExternalOutput"),
            "ds2": nc.dram_tensor("ds2", [128, 64, 130], _ddt,
                                  kind="ExternalOutput"),
        }
    const_d = {}
    for name, (shape, kind) in _const_specs().items():
        dt_ = _sdt() if kind == "stage" else F32
        const_d[name] = nc.dram_tensor(name, shape, dt_, kind="ExternalInput")

    with tile.TileContext(nc) as tc:
        from contextlib import ExitStack
        with ExitStack() as ctx:
            cpool = ctx.enter_context(tc.tile_pool(name="consts", bufs=1))
            spool = ctx.enter_context(tc.tile_pool(name="stage", bufs=1))
            xpool = ctx.enter_context(tc.tile_pool(name="xin", bufs=3))
            wpool = ctx.enter_context(tc.tile_pool(name="work", bufs=2))
            rpool = ctx.enter_context(tc.tile_pool(name="ret", bufs=1))
            dpool = ctx.enter_context(tc.tile_pool(name="dram", bufs=1, space="DRAM"))
            fftps = ctx.enter_context(tc.tile_pool(name="fftps", bufs=3, space="PSUM"))
            convps = ctx.enter_context(tc.tile_pool(name="convps", bufs=3, space="PSUM"))
            miscps = ctx.enter_context(tc.tile_pool(name="miscps", bufs=2, space="PSUM"))

            # critical-path consts (FFT G matrices + conv1 weights) load
            # first on the sync queue; everything else goes on the gpsimd
            # queue so image-0's FFT isn't stuck behind ~1MB of decoder
            # weights.
            crit = ("gre", "gim", "gimn", "w1l")
            cs = {}
            for name, (shape, kind) in _const_specs().items():
                dt_ = _sdt() if kind == "stage" else F32
                t = cpool.tile(shape, dt_, name=f"c_{name}")
                if name in crit:
                    nc.sync.dma_start(out=t, in_=const_d[name].ap())
                cs[name] = t

            # fixed stage buffers (all single-buffered; WAR deps order reuse)
            strip1 = spool.tile([128, 2, 32, 256], sdt, name="strip1")
            strip2 = spool.tile([128, 64, 130], sdt, name="strip2")
            strip3a = spool.tile([128, 64, 66], sdt, name="strip3a")
            strip3b = spool.tile([64, 64, 66], sdt, name="strip3b")
            xpooled1 = spool.tile([128, 64, 130], sdt, name="xpooled1")
            xpB1 = spool.tile([64, 64, 130], sdt, name="xpB1")
            h1X = spool.tile([64, 64, 130], sdt, name="h1X")
            xpooled2 = spool.tile([128, 64, 66], sdt, name="xpooled2")
            xpB2 = spool.tile([64, 64, 66], sdt, name="xpB2")
            h2buf = spool.tile([64, 64, 66], sdt, name="h2buf")
            xm = spool.tile([128, 2, 258], sdt, name="xm")
            qacc = spool.tile([128, 8], F32, name="qacc")
            qT = spool.tile([128, b_loc], F32, name="qT")
            xmd = dpool.tile([258, 258], sdt, name="xmd")

            for t in (strip1, strip2, strip3a, strip3b, xpooled1, xpooled2):
                nc.vector.memset(t, 0.0)
            nc.vector.memset(xm, 0.0)
            nc.vector.memset(strip1[0:1], 1.0)  # conv1 bias row (ones)
            zrow = cpool.tile([1, 2, 258], sdt, name="zrow")
            nc.vector.memset(zrow, 0.0)
            nc.sync.dma_start(  # xmd pad rows 0, 257 (cols padded per-write)
                out=dramap(xmd, 0, [[1, 1], [257 * 258, 2], [1, 258]]),
                in_=zrow)

            # ---------------- per-image pipeline helpers ----------------
            def load_x(img):
                t = xpool.tile([128, 2, 256], fft_in_dt, name="x_sb",
                               tag="x_sb")
                nc.gpsimd.dma_start(
                    out=t,
                    in_=dramap(x_in.ap(), img * 65536,
                               [[512, 128], [256, 2], [1, 256]]))
                return t

            def fft_step1(x_sb):
                """step1: yts[(nm, mt)] sbuf bf16 tiles [128(x), 256(u)]."""
                yts = {}
                for mt in range(2):
                    for nm, rt in (("re", "gre"), ("im", "gim")):
                        ps = fftps.tile([128, 256], F32, name="ps_yt",
                                        tag="fft")
                        n_mm = 2 * NSPLIT
                        i = 0
                        for kt in range(2):
                            for sp in range(NSPLIT):
                                nc.tensor.matmul(
                                    ps,
                                    x_sb[:, kt, mt * 128:(mt + 1) * 128],
                                    cs[rt][:, kt, sp, :],
                                    start=(i == 0), stop=(i == n_mm - 1))
                                i += 1
                        sb = wpool.tile([128, 256], fft_in_dt,
                                        name=f"yt{nm}{mt}", tag=f"yt{nm}{mt}")
                        nc.scalar.copy(sb, ps)
                        yts[(nm, mt)] = sb
                return yts

            def fft_step2_mag(yts):
                """step2 + magnitude -> xm [128, 2, 258] (padded cols)."""
                for mt in range(2):
                    zre = fftps.tile([128, 256], F32, name="ps_zre", tag="fft")
                    zim = fftps.tile([128, 256], F32, name="ps_zim", tag="fft")
                    for out_ps, combos in (
                        (zre, [("re", "gre"), ("im", "gimn")]),
                        (zim, [("re", "gim"), ("im", "gre")]),
                    ):
                        n_mm = 4 * NSPLIT
                        i = 0
                        for nm, rt in combos:
                            for kt in range(2):
                                for sp in range(NSPLIT):
                                    nc.tensor.matmul(
                                        out_ps,
                                        yts[(nm, kt)][:, mt * 128:(mt + 1) * 128],
                                        cs[rt][:, kt, sp, :],
                                        start=(i == 0), stop=(i == n_mm - 1))
                                    i += 1
                    t1 = wpool.tile([128, 256], F32, name="mag1", tag="mag1")
                    t2 = wpool.tile([128, 256], F32, name="mag2", tag="mag2")
                    nc.scalar.square(t1, zre)
                    nc.scalar.square(t2, zim)
                    nc.vector.tensor_add(t1, t1, t2)
                    nc.scalar.sqrt(mk(xm, 0, 1, 128, [[1, 256]], mt * 258 + 1),
                                   t1)

            strip_qs = [nc.sync, nc.scalar, nc.gpsimd]

            def xm_to_dram_and_strips(hs_list=(0, 1)):
                # full 258-wide rows (pads included) -> contiguous-ish writes
                nc.gpsimd.dma_start(
                    out=dramap(xmd, 258,
                               [[258, 128], [128 * 258, 2], [1, 258]]),
                    in_=xm)
                # spread the six strip reads over three queues so their
                # issue latencies overlap (matters for the pipeline fill)
                for hs in hs_list:
                    for dx in range(3):
                        strip_qs[dx].dma_start(
                            out=mk(strip1, 1 + 6 * dx, 1, 6,
                                   [[256, 32], [1, 256]], hs * 8192),
                            in_=dramap(xmd, 33024 * hs + dx,
                                       [[258, 6], [1032, 32], [1, 256]]))

            def conv1_half(hs):
                for ch in range(16):
                    sg = 32 * hs + 2 * ch
                    ps = convps.tile([128, 512], F32, name="c1ps", tag="conv")
                    for sp in range(NSPLIT):
                        nc.tensor.matmul(
                            ps, cs["w1l"][:, 0, sp, :],
                            strip1[:, hs, 2 * ch:2 * ch + 2, :],
                            start=(sp == 0), stop=(sp == NSPLIT - 1))
                    # relu on ACT (bias is in K-row 18), x-pool on DVE
                    rt = wpool.tile([128, 2, 256], sdt, name="rt1", tag="rt1",
                                    bufs=3)
                    nc.scalar.activation(rt, ps, ActFn.Relu)
                    nc.vector.tensor_max(
                        mk(xpooled1, 0, 1, 128, [[130, 2], [1, 128]],
                           sg * 130 + 1),
                        mk(rt, 0, 1, 128, [[256, 2], [2, 128]], 0),
                        mk(rt, 0, 1, 128, [[256, 2], [2, 128]], 1))
                # per-half y-pool: align upper partitions, fold into h1X
                lo, n = hs * 32 * 130, 32 * 130
                nc.sync.dma_start(
                    out=mk(xpB1, 0, 1, 64, [[1, n]], lo),
                    in_=mk(xpooled1, 64, 1, 64, [[1, n]], lo))
                nc.vector.tensor_max(
                    mk(h1X, 0, 1, 64, [[1, n]], lo),
                    mk(xpooled1, 0, 1, 64, [[1, n]], lo),
                    mk(xpB1, 0, 1, 64, [[1, n]], lo))

            def fills2():
                # strip2 fills: slot s2 of j holds h1 row 2*s2+j-1;
                # h1X partitions 0..31 = even rows (slot=y/2), 32..63 = odd.
                for j, d0, ns, g, s0 in ((0, 1, 63, 32, 0), (1, 0, 64, 0, 0),
                                         (2, 0, 64, 32, 0), (3, 0, 63, 0, 1)):
                    nc.sync.dma_start(
                        out=mk(strip2, 32 * j, 1, 32, [[1, ns * 130]],
                               d0 * 130),
                        in_=mk(h1X, g, 1, 32, [[1, ns * 130]], s0 * 130))

            def conv2_half(half):
                for ch in range(8 * half, 8 * half + 8):
                    ps = convps.tile([128, 512], F32, name="c2ps", tag="conv")
                    i = 0
                    for dx in range(3):
                        for sp in range(NSPLIT):
                            nc.tensor.matmul(
                                ps, cs["w2l"][:, dx, sp, :],
                                mk(strip2, 0, 1, 128, [[130, 4], [1, 128]],
                                   4 * ch * 130 + dx),
                                start=(i == 0), stop=(i == 3 * NSPLIT - 1))
                            i += 1
                    rt2 = wpool.tile([128, 4, 128], sdt, name="rt2", tag="rt2",
                                     bufs=3)
                    nc.scalar.activation(rt2, ps, ActFn.Relu,
                                         bias=cs["cb2"][:, 0:1])
                    nc.vector.tensor_max(
                        mk(xpooled2, 0, 1, 128, [[66, 4], [1, 64]],
                           4 * ch * 66 + 1),
                        mk(rt2, 0, 1, 128, [[128, 4], [2, 64]], 0),
                        mk(rt2, 0, 1, 128, [[128, 4], [2, 64]], 1))
                # per-half align + fold into h2buf (strip3 fills happen
                # later, after the previous image's conv3 has consumed the
                # strips)
                lo, n = half * 32 * 66, 32 * 66
                nc.sync.dma_start(
                    out=mk(xpB2, 0, 1, 64, [[1, n]], lo),
                    in_=mk(xpooled2, 64, 1, 64, [[1, n]], lo))
                nc.vector.tensor_max(
                    mk(h2buf, 0, 1, 64, [[1, n]], lo),
                    mk(xpooled2, 0, 1, 64, [[1, n]], lo),
                    mk(xpB2, 0, 1, 64, [[1, n]], lo))

            def fills3():
                # strip3a: d=0 partitions hold row s-1, d=1 hold row s;
                # strip3b holds row s+1 (edge slots stay zero from init).
                for st, p0, d0, ns, s0 in (
                        (strip3a, 0, 1, 63, 0), (strip3a, 64, 0, 64, 0),
                        (strip3b, 0, 0, 63, 1)):
                    nc.sync.dma_start(
                        out=mk(st, p0, 1, 64, [[1, ns * 66]], d0 * 66),
                        in_=mk(h2buf, 0, 1, 64, [[1, ns * 66]], s0 * 66))

            def conv3_all(img):
                for ch in range(8):
                    ps = convps.tile([128, 512], F32, name="c3ps", tag="conv")
                    n_mm = 6 * NSPLIT
                    i = 0
                    for dx in range(3):
                        for w_, st3, pc in (("w3a", strip3a, 128),
                                            ("w3b", strip3b, 64)):
                            for sp in range(NSPLIT):
                                nc.tensor.matmul(
                                    ps, cs[w_][:, dx, sp, :],
                                    mk(st3, 0, 1, pc, [[66, 8], [1, 64]],
                                       8 * ch * 66 + dx),
                                    start=(i == 0), stop=(i == n_mm - 1))
                                i += 1
                    scr = wpool.tile([128, 512], F32, name="scr3", tag="scr3",
                                     bufs=2)
                    nc.scalar.activation(scr, ps, ActFn.Relu,
                                         bias=cs["cb3"][:, 0:1],
                                         accum_out=qacc[:, ch:ch + 1])
                nc.vector.reduce_sum(qT[:, img:img + 1], qacc, axis=AxX)

            # ---------------- software-pipelined image loop ----------------
            rep = int(os.environ.get("K_REP", "1"))
            loop_cm = tc.For_i(0, rep, 1) if rep > 1 else None
            if loop_cm is not None:
                loop_cm.__enter__()

            x_tiles = {0: load_x(0)}
            if b_loc > 1:
                x_tiles[1] = load_x(1)

            def load_misc_consts():
                for name, (shape, kind) in _const_specs().items():
                    if name not in crit:
                        nc.gpsimd.dma_start(out=cs[name],
                                            in_=const_d[name].ap())
            if os.environ.get("K_NOSKEW"):
                for i in range(b_loc):
                    yts = fft_step1(x_tiles[i])
                    fft_step2_mag(yts)
                    xm_to_dram_and_strips()
                    if i == 0:
                        load_misc_consts()
                    if i + 2 < b_loc:
                        x_tiles[i + 2] = load_x(i + 2)
                    conv1_half(0)
                    conv1_half(1)
                    fills2()
                    conv2_half(0)
                    conv2_half(1)
                    fills3()
                    conv3_all(i)
                    x_tiles.pop(i, None)
            else:
                # 3-deep skew: iteration i runs FFT(i+1), conv1(i),
                # conv2(i-1) and conv3(i-2), so every staging chain has at
                # least a full conv phase of matmuls to hide behind.
                yts = fft_step1(x_tiles[0])
                fft_step2_mag(yts)
                xm_to_dram_and_strips()
                load_misc_consts()
                for i in range(b_loc):
                    if i + 1 < b_loc:
                        yts = fft_step1(x_tiles[i + 1])
                    conv1_half(0)
                    conv1_half(1)
                    if i + 1 < b_loc:
                        fft_step2_mag(yts)
                        xm_to_dram_and_strips()
                    if i + 2 < b_loc:
                        x_tiles[i + 2] = load_x(i + 2)
                    if i >= 1:
                        conv2_half(0)
                        conv2_half(1)
                    fills2()
                    if i >= 2:
                        conv3_all(i - 2)
                    if i >= 1:
                        fills3()
                    x_tiles.pop(i, None)
                conv2_half(0)
                conv2_half(1)
                if b_loc >= 2:
                    conv3_all(b_loc - 2)
                fills3()
                conv3_all(b_loc - 1)

            # ---------------- retrieval (fp32) ----------------
            # key normalization -> knT [128, 400] (fp32)
            knT = rpool.tile([128, 400], F32, name="knT")
            ksq = rpool.tile([128, 4, 128], F32, name="ksq")
            nc.vector.tensor_mul(ksq, cs["keys"], cs["keys"])
            kss = rpool.tile([128, 4], F32, name="kss")
            nc.vector.reduce_sum(kss, ksq, axis=AxX)
            knm = rpool.tile([128, 4], F32, name="knm")
            nc.scalar.sqrt(knm, kss)
            nc.vector.tensor_scalar_max(knm, knm, 1e-12)
            kri = rpool.tile([128, 4], F32, name="kri")
            nc.vector.reciprocal(kri, knm)
            knrm = rpool.tile([128, 4, 128], F32, name="knrm")
            for c in range(4):
                nc.vector.tensor_scalar_mul(
                    knrm[:, c, :], cs["keys"][:, c, :], kri[:, c:c + 1])
            for c in range(4):
                pc = 128 if c < 3 else 16
                tp = miscps.tile([128, 128], F32, name="tp_kn", tag="misc")
                nc.tensor.transpose(
                    tp[:, :pc], knrm[:pc, c, :], cs["ident"][:pc, :pc])
                nc.scalar.copy(knT[:, c * 128:c * 128 + pc], tp[:, :pc])

            if qdbg_d is not None:
                nc.sync.dma_start(out=qdbg_d.ap(), in_=qT)
            if hdbg_d is not None:
                # dump last-image staging tiles raw (bf16)
                for nm_, src_ in (("dxm", xm), ("dh1", h1X), ("dh2", h2buf),
                                  ("ds2", strip2)):
                    nc.sync.dma_start(out=hdbg_d[nm_].ap(), in_=src_)
            bl = b_loc
            simps = miscps.tile([bl, 400], F32, name="simps", tag="misc")
            nc.tensor.matmul(simps, qT, knT, start=True, stop=True)
            gram = miscps.tile([bl, bl], F32, name="gram", tag="misc")
            nc.tensor.matmul(gram, qT, qT, start=True, stop=True)
            gd = rpool.tile([bl, bl], F32, name="gd")
            nc.vector.tensor_mul(gd, gram, cs["ident"][:bl, :bl])
            q2 = rpool.tile([bl, 1], F32, name="q2")
            nc.vector.reduce_sum(q2, gd, axis=AxX)
            qn = rpool.tile([bl, 1], F32, name="qn")
            nc.scalar.sqrt(qn, q2)
            nc.vector.tensor_scalar_max(qn, qn, 1e-12)
            rq = rpool.tile([bl, 1], F32, name="rq")
            nc.vector.reciprocal(rq, qn)
            sim = rpool.tile([bl, 400], F32, name="sim")
            nc.vector.tensor_scalar_mul(sim, simps, rq[:, 0:1])

            cur = rpool.tile([bl, 400], F32, name="cur")
            nc.vector.tensor_copy(cur, sim)
            m1 = rpool.tile([bl, 1], F32, name="m1")
            nc.vector.reduce_max(m1, sim, axis=AxX)
            msk = rpool.tile([bl, 400], F32, name="msk")
            mk_ = m1
            for it in range(4):
                nc.vector.tensor_scalar(msk, cur, mk_[:, 0:1], None,
                                        op0=AluOp.is_ge)
                nc.vector.scalar_tensor_tensor(cur, msk, -1e30, cur,
                                               op0=AluOp.mult, op1=AluOp.add)
                nm_ = rpool.tile([bl, 1], F32, name=f"mk{it}")
                nc.vector.reduce_max(nm_, cur, axis=AxX)
                mk_ = nm_
            m5 = mk_
            nc.vector.tensor_scalar(msk, sim, m5[:, 0:1], None, op0=AluOp.is_ge)
            m1n = rpool.tile([bl, 1], F32, name="m1n")
            nc.vector.tensor_scalar_mul(m1n, m1, -1.0)
            es = rpool.tile([bl, 400], F32, name="es")
            nc.scalar.activation(es, sim, ActFn.Exp, bias=m1n[:, 0:1])
            ew = rpool.tile([bl, 400], F32, name="ew")
            nc.vector.tensor_mul(ew, es, msk)
            zs = rpool.tile([bl, 1], F32, name="zs")
            nc.vector.reduce_sum(zs, ew, axis=AxX)
            rz = rpool.tile([bl, 1], F32, name="rz")
            nc.vector.reciprocal(rz, zs)
            nc.vector.tensor_scalar_mul(ew, ew, rz[:, 0:1])

            eT = rpool.tile([128, 4, bl], F32, name="eT")
            for c in range(4):
                pc = 128 if c < 3 else 16
                tp = miscps.tile([128, bl], F32, name="tp_e", tag="misc")
                nc.tensor.transpose(tp[:pc, :], ew[:, c * 128:c * 128 + pc],
                                    cs["ident"][:bl, :bl])
                nc.scalar.copy(eT[:pc, c, :], tp[:pc, :])

            memps = miscps.tile([128, bl], F32, name="memps", tag="misc")
            for c in range(4):
                pc = 128 if c < 3 else 16
                nc.tensor.matmul(memps, cs["vals"][:pc, c, :], eT[:pc, c, :],
                                 start=(c == 0), stop=(c == 3))
            memT = rpool.tile([128, bl], F32, name="memT")
            nc.scalar.copy(memT, memps)

            h1T = rpool.tile([128, 2, bl], F32, name="h1T")
            for mt in range(2):
                ps = miscps.tile([128, bl], F32, name="d1ps", tag="misc")
                nc.tensor.matmul(ps, cs["w1d"][:, mt * 128:(mt + 1) * 128],
                                 memT, start=True, stop=True)
                nc.scalar.activation(h1T[:, mt, :], ps, ActFn.Relu,
                                     bias=cs["b1d"][:, mt:mt + 1])
            h2T = rpool.tile([128, 4, bl], F32, name="h2T")
            for mt in range(4):
                ps = miscps.tile([128, bl], F32, name="d2ps", tag="misc")
                for kt in range(2):
                    nc.tensor.matmul(ps, cs["w2d"][:, kt, mt, :], h1T[:, kt, :],
                                     start=(kt == 0), stop=(kt == 1))
                nc.scalar.activation(h2T[:, mt, :], ps, ActFn.Relu,
                                     bias=cs["b2d"][:, mt:mt + 1])
            ops = miscps.tile([bl, 16], F32, name="outps", tag="misc")
            for c in range(4):
                nc.tensor.matmul(ops, h2T[:, c, :], cs["w3d"][:, c, :],
                                 start=(c == 0), stop=False)
            nc.tensor.matmul(ops, cs["ones1"][:, :bl], cs["b3row"],
                             start=False, stop=True)
            out_sb = rpool.tile([bl, 16], F32, name="out_sb")
            nc.scalar.copy(out_sb, ops)
            nc.sync.dma_start(out=out_d.ap(), in_=out_sb)
            if loop_cm is not None:
                loop_cm.__exit__(None, None, None)

    nc.compile()
    return nc


# ---------------------------------------------------------------------------
# host entry
# ---------------------------------------------------------------------------
_NC_CACHE = {}


def _get_nc(b_loc):
    key = (b_loc, STAGE, os.environ.get("K_REP", "1"),
           os.environ.get("K_NOSKEW"), os.environ.get("K_SYNCQ"),
           os.environ.get("K_DBGQ"), os.environ.get("K_DBGH"))
    if key not in _NC_CACHE:
        _NC_CACHE[key] = build_nc(b_loc)
    return _NC_CACHE[key]


def _pack_x(x_shard):
    b = x_shard.shape[0]
    xr = np.ascontiguousarray(
        x_shard.reshape(b, 2, 128, 256).transpose(0, 2, 1, 3)).astype(np.float32)
    return xr.astype(_np_sdt())


def kernel(**inputs):
    x = np.asarray(inputs["x"], np.float32)
    # jnp.fft.fftshift also shifts the batch axis: output b uses x[(b+64)%128]
    xp = np.roll(x, -64, axis=0)
    consts = _host_consts(inputs)

    b_loc = B // N_CORES
    nc = _get_nc(b_loc)

    in_maps = []
    for c in range(N_CORES):
        m = dict(consts)
        m["x_in"] = _pack_x(xp[c * b_loc:(c + 1) * b_loc])
        in_maps.append(m)

    kwargs = {}
    if os.environ.get("K_TRACE"):
        kwargs["trace"] = True
    res = run_bass_kernel_spmd(nc, in_maps, core_ids=list(range(N_CORES)),
                               **kwargs)
    global LAST_RESULTS
    LAST_RESULTS = res
    out = np.concatenate([r["out"] for r in res.results], axis=0)
    return out.reshape(B, 1, 4, 4).astype(np.float32)


LAST_RESULTS = None


if __name__ == "__main__":
    build_nc(int(os.environ.get("K_BLOC", "1")))
    print("built ok")


# revision 43
# speedup vs baseline: 1.3649x; 1.0134x over previous
"""Trainium2 Bass kernel for nn_FFTMemAutoEncoderBranch (retrieval_knn).

Data-parallel over batch: 8 cores x 16 images, no cross-core communication.

Numerics: the problem's top-5 retrieval runs on near-identical queries (white
-noise FFT magnitudes), with 5th/6th similarity gaps down to 7e-6 -- so conv
weights and DFT matrices must act at ~fp32 fidelity while activations tolerate
bf16. Scheme (validated against the reference on HW, 0/128 top-5 flips):
  - activations/staging in bf16
  - every stationary operand (DFT matrices G, conv weights) is split
    W = hi + lo into two bf16 matmuls accumulating in fp32 PSUM
  - retrieval + decoder in fp32

Performance structure (v2): the v1 kernel ran image-major with serial staging,
leaving the PE idle ~80us/image (trace: 1.25ms of gaps in a 3.2ms span, HAM
re-throttling the PE to 1.2GHz for ~85% of the run) and moving ~100MB/core of
SBUF<->SBUF staging in 256-512B DMA descriptors. v2:
  - software-pipelines images with a one-iteration skew: PE order is
    [FFTs1(i+1) | conv1(i) | FFTs2(i+1) | conv2(i) | conv3(i)], so every
    staging chain (mag->DRAM bounce->strip1; pool->align->fold->strip fills)
    runs in the shadow of ~30-90us of matmuls from the neighboring image.
  - staging tiles use pitch-matched padded rows (130-wide for conv2 strips,
    66-wide for conv3) so each strip fill is a handful of multi-KB-contiguous
    descriptors instead of thousands of 256B ones.
  - conv1 folds bias into a 19th K-row (rhs row of ones) and fuses
    relu+x-pool into one DVE scalar_tensor_tensor reading PSUM directly.
  - pool/align/fold/fill chains run per half-image so conv(i) chunk k's rhs
    is staged while chunks k-1 of the same image still run.

Per image: FFT2 as DFT matmuls (z = G x G^T, G = roll(F,128,0)/16, fftshift+
ortho folded in; batch roll done on host), conv1 strips via a DRAM bounce of
the padded 258x258 magnitude image, conv2/conv3 strips via parity-split SBUF
copies, maxpools on DVE, conv3 relu accumulating into q via ACT accum_out.
Retrieval: fp32 sim, top-5 threshold via 5x(reduce_max+mask), masked stable
softmax, mem = values^T @ e^T; decoder collapsed to 3 dense matmuls.
"""

import os
import sys
import numpy as np

for _p in ("/opt/trn_rl_repo", "/root/.axon_site/_ro/trn_rl_repo"):
    if os.path.isdir(_p) and _p not in sys.path:
        sys.path.append(_p)

import concourse.bass as bass
import concourse.mybir as mybir
import concourse.tile as tile
from concourse import bacc
from concourse.bass_utils import run_bass_kernel_spmd

F32 = mybir.dt.float32
BF16 = mybir.dt.bfloat16

N_CORES = 8
B = 128
H = 256

# STAGE: "bf16" (hi/lo-split weights, bf16 activations) | "f32" (all fp32)
STAGE = os.environ.get("K_STAGE", "bf16")
NSPLIT = 2 if STAGE == "bf16" else 1

AluOp = mybir.AluOpType
ActFn = mybir.ActivationFunctionType
AxX = mybir.AxisListType.X


def _sdt():
    return BF16 if STAGE == "bf16" else F32


def _np_sdt():
    if STAGE == "bf16":
        import ml_dtypes
        return ml_dtypes.bfloat16
    return np.float32


# ---------------------------------------------------------------------------
# host-side constant construction
# ---------------------------------------------------------------------------
def _pack2(m):  # [256, N] -> [128, 2, N]
    return np.ascontiguousarray(m.reshape(2, 128, -1).transpose(1, 0, 2))


def _fft_consts():
    k = np.arange(H)
    F = np.exp(-2j * np.pi * np.outer(k, k) / H) / 16.0
    G = np.roll(F, H // 2, axis=0)
    GT = G.T.copy()
    out = {}
    for name, m in (("gre", GT.real), ("gim", GT.imag), ("gimn", -GT.imag)):
        m = m.astype(np.float32)
        if NSPLIT == 1:
            out[name] = _pack2(m)[:, :, None, :]  # [128, 2, 1, 256] f32
        else:
            sdt = _np_sdt()
            hi32 = m.astype(sdt).astype(np.float32)
            lo = (m - hi32).astype(sdt)
            hi = m.astype(sdt)
            # [128, 2, 2, 256]: (part-of-256-rows, half, split, col)
            out[name] = np.stack([_pack2(hi), _pack2(lo)], axis=2)
    return out


T1ORD = (0, 2, 1, 3)  # conv1 M block -> strip row offset t; so that y-pool
# pairs (t0,t1),(t2,t3) become max(partitions 0:64, partitions 64:128)


def _conv1_lhsT(we1):
    # K = (dx, j) packed on partitions 0..17, p = dx*6 + j (dx-major so each
    # strip1 fill DMA writes a contiguous partition block). K is padded to
    # 128 with zero rows so every matmul uses the full PE array (the HAM
    # activity monitor demotes the clock when row-group-0-only matmuls mix
    # with full-array ones).
    out = np.zeros((128, 1, 128), np.float32)
    for dx in range(3):
        for m in range(128):
            t, co = T1ORD[m // 32], m % 32
            for j in range(6):
                if 0 <= j - t <= 2:
                    out[dx * 6 + j, 0, m] = we1[co, 0, j - t, dx]
    return out


def _conv2_lhsT(we2):
    # K layout j-major: k = j*32 + ci (each strip2 fill writes a contiguous
    # partition block)
    out = np.zeros((128, 3, 128), np.float32)
    for dx in range(3):
        for m in range(128):
            t, co = m // 64, m % 64
            for k in range(128):
                ci, j = k % 32, k // 32
                if 0 <= j - t <= 2:
                    out[k, dx, m] = we2[co, ci, j - t, dx]
    return out


def _conv3_lhsT(we3):
    # K layout d-major: k = d*64 + ci
    A = np.zeros((128, 3, 128), np.float32)
    Bm = np.zeros((64, 3, 128), np.float32)
    for dx in range(3):
        for k in range(128):
            ci, d = k % 64, k // 64
            A[k, dx, :] = we3[:, ci, d, dx]
        for ci in range(64):
            Bm[ci, dx, :] = we3[:, ci, 2, dx]
    return A, Bm


def _wsplit(w):
    """[P, D, N] -> [P, D, NSPLIT, N] staging dtype (hi, lo)."""
    sdt = _np_sdt()
    if NSPLIT == 1:
        return w[:, :, None, :].astype(np.float32)
    hi32 = w.astype(sdt).astype(np.float32)
    lo = (w - hi32).astype(sdt)
    return np.stack([w.astype(sdt), lo], axis=2)


def _decoder_mats(wd1, bd1, wd2, bd2, wd3, bd3):
    W1 = np.zeros((128, 256), np.float32)
    for c in range(64):
        for i in range(2):
            for j in range(2):
                W1[:, c * 4 + i * 2 + j] = wd1[:, c, i + 1, j + 1]
    b1 = np.repeat(bd1, 4).astype(np.float32)

    W2 = np.zeros((256, 512), np.float32)
    for c in range(64):
        for ii in range(2):
            for jj in range(2):
                f = c * 4 + ii * 2 + jj
                for c2 in range(32):
                    for y in range(4):
                        ky = y + 1 - 2 * ii
                        if not (0 <= ky <= 3):
                            continue
                        for x in range(4):
                            kx = x + 1 - 2 * jj
                            if 0 <= kx <= 3:
                                W2[f, c2 * 16 + y * 4 + x] = wd2[c, c2, ky, kx]
    b2 = np.repeat(bd2, 16).astype(np.float32)

    W3 = np.zeros((512, 16), np.float32)
    for c2 in range(32):
        for y in range(4):
            for x in range(4):
                g = c2 * 16 + y * 4 + x
                for oy in range(4):
                    ky = y - oy + 1
                    if not (0 <= ky <= 2):
                        continue
                    for ox in range(4):
                        kx = x - ox + 1
                        if 0 <= kx <= 2:
                            W3[g, oy * 4 + ox] = wd3[0, c2, ky, kx]
    b3 = np.full((16,), float(np.asarray(bd3).reshape(-1)[0]), np.float32)
    return W1, b1, W2, b2, W3, b3


def _host_consts(inputs):
    w3a, w3b = _conv3_lhsT(np.asarray(inputs["we3"], np.float32))
    W1, b1, W2, b2, W3, b3 = _decoder_mats(
        np.asarray(inputs["wd1"], np.float32), np.asarray(inputs["bd1"], np.float32),
        np.asarray(inputs["wd2"], np.float32), np.asarray(inputs["bd2"], np.float32),
        np.asarray(inputs["wd3"], np.float32), np.asarray(inputs["bd3"], np.float32))

    keys = np.asarray(inputs["keys"], np.float32)
    values = np.asarray(inputs["values"], np.float32)
    keys_p = np.ones((512, 128), np.float32)
    keys_p[:400] = keys
    values_p = np.zeros((512, 128), np.float32)
    values_p[:400] = values

    c = dict(_fft_consts())
    c.update({
        "w1l": _wsplit(_conv1_lhsT(np.asarray(inputs["we1"], np.float32))),
        "cb1": np.tile(np.asarray(inputs["be1"], np.float32), 4).reshape(128, 1),
        "w2l": _wsplit(_conv2_lhsT(np.asarray(inputs["we2"], np.float32))),
        "w3a": _wsplit(w3a), "w3b": _wsplit(w3b),
        "cb2": np.tile(np.asarray(inputs["be2"], np.float32), 2).reshape(128, 1),
        "cb3": np.asarray(inputs["be3"], np.float32).reshape(128, 1),
        "keys": np.ascontiguousarray(keys_p.reshape(4, 128, 128).transpose(1, 0, 2)),
        "vals": np.ascontiguousarray(values_p.reshape(4, 128, 128).transpose(1, 0, 2)),
        "ident": np.eye(128, dtype=np.float32),
        "w1d": W1,
        "w2d": np.ascontiguousarray(W2.reshape(2, 128, 4, 128).transpose(1, 0, 2, 3)),
        "w3d": np.ascontiguousarray(W3.reshape(4, 128, 16).transpose(1, 0, 2)),
        "b1d": np.ascontiguousarray(b1.reshape(2, 128).T),
        "b2d": np.ascontiguousarray(b2.reshape(4, 128).T),
        "b3row": b3.reshape(1, 16),
        "ones1": np.ones((1, 16), np.float32),
    })
    return c


def _const_specs():
    s = "stage"
    return {
        "gre": ([128, 2, NSPLIT, 256], s), "gim": ([128, 2, NSPLIT, 256], s),
        "gimn": ([128, 2, NSPLIT, 256], s),
        "w1l": ([128, 1, NSPLIT, 128], s), "w2l": ([128, 3, NSPLIT, 128], s),
        "w3a": ([128, 3, NSPLIT, 128], s), "w3b": ([64, 3, NSPLIT, 128], s),
        "cb1": ([128, 1], "f32"), "cb2": ([128, 1], "f32"),
        "cb3": ([128, 1], "f32"),
        "keys": ([128, 4, 128], "f32"), "vals": ([128, 4, 128], "f32"),
        "ident": ([128, 128], "f32"),
        "w1d": ([128, 256], "f32"), "w2d": ([128, 2, 4, 128], "f32"),
        "w3d": ([128, 4, 16], "f32"),
        "b1d": ([128, 2], "f32"), "b2d": ([128, 4], "f32"),
        "b3row": ([1, 16], "f32"), "ones1": ([1, 16], "f32"),
    }


def mk(t, poff, pstep, pcount, fdims, foff=0):
    """Manual AP on tile t (element units; partition pitch from the tile AP)."""
    pitch = t.ap[0][0]
    dims = [[pstep * pitch, pcount]] + [list(d) for d in fdims]
    return bass.AP(t.tensor, t.offset + poff * pitch + foff, dims)


def dramap(t, off, dims):
    return bass.AP(t.tensor, t.offset + off, [list(d) for d in dims])


# ---------------------------------------------------------------------------
# kernel builder
# ---------------------------------------------------------------------------
def build_nc(b_loc=16):
    sdt = _sdt()
    fft_in_dt = BF16 if STAGE == "bf16" else F32
    nc = bacc.Bacc("TRN2", target_bir_lowering=False, debug=False)

    x_in = nc.dram_tensor("x_in", [b_loc, 128, 2, 256], fft_in_dt,
                          kind="ExternalInput")
    out_d = nc.dram_tensor("out", [b_loc, 16], F32, kind="ExternalOutput")
    qdbg_d = (nc.dram_tensor("qdbg", [128, b_loc], F32, kind="ExternalOutput")
              if os.environ.get("K_DBGQ") else None)
    hdbg_d = None
    if os.environ.get("K_DBGH"):
        _ddt = _sdt()
        hdbg_d = {
            "dxm": nc.dram_tensor("dxm", [128, 2, 258], _ddt,
                                  kind="ExternalOutput"),
            "dh1": nc.dram_tensor("dh1", [64, 64, 130], _ddt,
                                  kind="ExternalOutput"),
            "dh2": nc.dram_tensor("dh2", [64, 64, 66], _ddt,
                                  kind="ExternalOutput"),
            "ds2": nc.dram_tensor("ds2", [128, 64, 130], _ddt,
                                  kind="ExternalOutput"),
        }
    const_d = {}
    for name, (shape, kind) in _const_specs().items():
        dt_ = _sdt() if kind == "stage" else F32
        const_d[name] = nc.dram_tensor(name, shape, dt_, kind="ExternalInput")

    with tile.TileContext(nc) as tc:
        from contextlib import ExitStack
        with ExitStack() as ctx:
            cpool = ctx.enter_context(tc.tile_pool(name="consts", bufs=1))
            spool = ctx.enter_context(tc.tile_pool(name="stage", bufs=1))
            xpool = ctx.enter_context(tc.tile_pool(name="xin", bufs=3))
            wpool = ctx.enter_context(tc.tile_pool(name="work", bufs=2))
            rpool = ctx.enter_context(tc.tile_pool(name="ret", bufs=1))
            dpool = ctx.enter_context(tc.tile_pool(name="dram", bufs=1, space="DRAM"))
            fftps = ctx.enter_context(tc.tile_pool(name="fftps", bufs=3, space="PSUM"))
            convps = ctx.enter_context(tc.tile_pool(name="convps", bufs=3, space="PSUM"))
            miscps = ctx.enter_context(tc.tile_pool(name="miscps", bufs=2, space="PSUM"))

            # critical-path consts (FFT G matrices + conv1 weights) load
            # first on the sync queue; everything else goes on the gpsimd
            # queue so image-0's FFT isn't stuck behind ~1MB of decoder
            # weights.
            crit = ("gre", "gim", "gimn", "w1l")
            cs = {}
            for name, (shape, kind) in _const_specs().items():
                dt_ = _sdt() if kind == "stage" else F32
                t = cpool.tile(shape, dt_, name=f"c_{name}")
                if name in crit:
                    nc.sync.dma_start(out=t, in_=const_d[name].ap())
                cs[name] = t

            # fixed stage buffers (all single-buffered; WAR deps order reuse)
            strip1 = spool.tile([128, 2, 32, 256], sdt, name="strip1")
            strip2 = spool.tile([128, 64, 130], sdt, name="strip2")
            strip3a = spool.tile([128, 64, 66], sdt, name="strip3a")
            strip3b = spool.tile([64, 64, 66], sdt, name="strip3b")
            xpooled1 = spool.tile([128, 64, 130], sdt, name="xpooled1")
            xpB1 = spool.tile([64, 64, 130], sdt, name="xpB1")
            h1X = spool.tile([64, 64, 130], sdt, name="h1X")
            xpooled2 = spool.tile([128, 64, 66], sdt, name="xpooled2")
            xpB2 = spool.tile([64, 64, 66], sdt, name="xpB2")
            h2buf = spool.tile([64, 64, 66], sdt, name="h2buf")
            xm = spool.tile([128, 2, 258], sdt, name="xm")
            qacc = spool.tile([128, 8], F32, name="qacc")
            qT = spool.tile([128, b_loc], F32, name="qT")
            xmd = dpool.tile([258, 258], sdt, name="xmd")

            nc.vector.memset(xm, 0.0)
            for t in (strip1, xpooled1, strip2, xpooled2, strip3a, strip3b):
                nc.vector.memset(t, 0.0)
            zrow = cpool.tile([1, 2, 258], sdt, name="zrow")
            nc.vector.memset(zrow, 0.0)
            nc.sync.dma_start(  # xmd pad rows 0, 257 (cols padded per-write)
                out=dramap(xmd, 0, [[1, 1], [257 * 258, 2], [1, 258]]),
                in_=zrow)

            # ---------------- per-image pipeline helpers ----------------
            def load_x(img):
                t = xpool.tile([128, 2, 256], fft_in_dt, name="x_sb",
                               tag="x_sb")
                nc.gpsimd.dma_start(
                    out=t,
                    in_=dramap(x_in.ap(), img * 65536,
                               [[512, 128], [256, 2], [1, 256]]))
                return t

            def fft_step1(x_sb):
                """step1: yts[(nm, mt)] sbuf bf16 tiles [128(x), 256(u)]."""
                yts = {}
                for mt in range(2):
                    for nm, rt in (("re", "gre"), ("im", "gim")):
                        ps = fftps.tile([128, 256], F32, name="ps_yt",
                                        tag="fft")
                        n_mm = 2 * NSPLIT
                        i = 0
                        for kt in range(2):
                            for sp in range(NSPLIT):
                                nc.tensor.matmul(
                                    ps,
                                    x_sb[:, kt, mt * 128:(mt + 1) * 128],
                                    cs[rt][:, kt, sp, :],
                                    start=(i == 0), stop=(i == n_mm - 1))
                                i += 1
                        sb = wpool.tile([128, 256], fft_in_dt,
                                        name=f"yt{nm}{mt}", tag=f"yt{nm}{mt}")
                        nc.scalar.copy(sb, ps)
                        yts[(nm, mt)] = sb
                return yts

            def fft_step2_mag(yts):
                """step2 + magnitude -> xm [128, 2, 258] (padded cols)."""
                for mt in range(2):
                    zre = fftps.tile([128, 256], F32, name="ps_zre", tag="fft")
                    zim = fftps.tile([128, 256], F32, name="ps_zim", tag="fft")
                    for out_ps, combos in (
                        (zre, [("re", "gre"), ("im", "gimn")]),
                        (zim, [("re", "gim"), ("im", "gre")]),
                    ):
                        n_mm = 4 * NSPLIT
                        i = 0
                        for nm, rt in combos:
                            for kt in range(2):
                                for sp in range(NSPLIT):
                                    nc.tensor.matmul(
                                        out_ps,
                                        yts[(nm, kt)][:, mt * 128:(mt + 1) * 128],
                                        cs[rt][:, kt, sp, :],
                                        start=(i == 0), stop=(i == n_mm - 1))
                                    i += 1
                    t1 = wpool.tile([128, 256], F32, name="mag1", tag="mag1")
                    t2 = wpool.tile([128, 256], F32, name="mag2", tag="mag2")
                    nc.scalar.square(t1, zre)
                    nc.scalar.square(t2, zim)
                    nc.gpsimd.tensor_add(t1, t1, t2)
                    nc.scalar.sqrt(mk(xm, 0, 1, 128, [[1, 256]], mt * 258 + 1),
                                   t1)

            strip_qs = [nc.sync, nc.scalar, nc.gpsimd]

            def xm_to_dram_and_strips(hs_list=(0, 1)):
                # full 258-wide rows (pads included) -> contiguous-ish writes
                nc.gpsimd.dma_start(
                    out=dramap(xmd, 258,
                               [[258, 128], [128 * 258, 2], [1, 258]]),
                    in_=xm)
                # spread the six strip reads over three queues so their
                # issue latencies overlap (matters for the pipeline fill)
                for hs in hs_list:
                    for dx in range(3):
                        strip_qs[dx].dma_start(
                            out=mk(strip1, 6 * dx, 1, 6,
                                   [[256, 32], [1, 256]], hs * 8192),
                            in_=dramap(xmd, 33024 * hs + dx,
                                       [[258, 6], [1032, 32], [1, 256]]))

            def conv1_half(hs):
                for ch in range(16):
                    sg = 32 * hs + 2 * ch
                    ps = convps.tile([128, 512], F32, name="c1ps", tag="conv")
                    for sp in range(NSPLIT):
                        nc.tensor.matmul(
                            ps, cs["w1l"][:, 0, sp, :],
                            strip1[:, hs, 2 * ch:2 * ch + 2, :],
                            start=(sp == 0), stop=(sp == NSPLIT - 1))
                    # relu+bias on ACT, x-pool on DVE
                    rt = wpool.tile([128, 2, 256], sdt, name="rt1", tag="rt1",
                                    bufs=3)
                    nc.scalar.activation(rt, ps, ActFn.Relu,
                                         bias=cs["cb1"][:, 0:1])
                    nc.vector.tensor_max(
                        mk(xpooled1, 0, 1, 128, [[130, 2], [1, 128]],
                           sg * 130 + 1),
                        mk(rt, 0, 1, 128, [[256, 2], [2, 128]], 0),
                        mk(rt, 0, 1, 128, [[256, 2], [2, 128]], 1))
                # per-half y-pool: align upper partitions, fold into h1X
                lo, n = hs * 32 * 130, 32 * 130
                nc.sync.dma_start(
                    out=mk(xpB1, 0, 1, 64, [[1, n]], lo),
                    in_=mk(xpooled1, 64, 1, 64, [[1, n]], lo))
                nc.vector.tensor_max(
                    mk(h1X, 0, 1, 64, [[1, n]], lo),
                    mk(xpooled1, 0, 1, 64, [[1, n]], lo),
                    mk(xpB1, 0, 1, 64, [[1, n]], lo))

            def fills2():
                # strip2 fills: slot s2 of j holds h1 row 2*s2+j-1;
                # h1X partitions 0..31 = even rows (slot=y/2), 32..63 = odd.
                for j, d0, ns, g, s0 in ((0, 1, 63, 32, 0), (1, 0, 64, 0, 0),
                                         (2, 0, 64, 32, 0), (3, 0, 63, 0, 1)):
                    nc.sync.dma_start(
                        out=mk(strip2, 32 * j, 1, 32, [[1, ns * 130]],
                               d0 * 130),
                        in_=mk(h1X, g, 1, 32, [[1, ns * 130]], s0 * 130))

            def conv2_half(half):
                for ch in range(8 * half, 8 * half + 8):
                    ps = convps.tile([128, 512], F32, name="c2ps", tag="conv")
                    i = 0
                    for dx in range(3):
                        for sp in range(NSPLIT):
                            nc.tensor.matmul(
                                ps, cs["w2l"][:, dx, sp, :],
                                mk(strip2, 0, 1, 128, [[130, 4], [1, 128]],
                                   4 * ch * 130 + dx),
                                start=(i == 0), stop=(i == 3 * NSPLIT - 1))
                            i += 1
                    rt2 = wpool.tile([128, 4, 128], sdt, name="rt2", tag="rt2",
                                     bufs=3)
                    nc.scalar.activation(rt2, ps, ActFn.Relu,
                                         bias=cs["cb2"][:, 0:1])
                    nc.vector.tensor_max(
                        mk(xpooled2, 0, 1, 128, [[66, 4], [1, 64]],
                           4 * ch * 66 + 1),
                        mk(rt2, 0, 1, 128, [[128, 4], [2, 64]], 0),
                        mk(rt2, 0, 1, 128, [[128, 4], [2, 64]], 1))
                # per-half align + fold into h2buf (strip3 fills happen
                # later, after the previous image's conv3 has consumed the
                # strips)
                lo, n = half * 32 * 66, 32 * 66
                nc.sync.dma_start(
                    out=mk(xpB2, 0, 1, 64, [[1, n]], lo),
                    in_=mk(xpooled2, 64, 1, 64, [[1, n]], lo))
                nc.vector.tensor_max(
                    mk(h2buf, 0, 1, 64, [[1, n]], lo),
                    mk(xpooled2, 0, 1, 64, [[1, n]], lo),
                    mk(xpB2, 0, 1, 64, [[1, n]], lo))

            def fills3():
                # strip3a: d=0 partitions hold row s-1, d=1 hold row s;
                # strip3b holds row s+1 (edge slots stay zero from init).
                for st, p0, d0, ns, s0 in (
                        (strip3a, 0, 1, 63, 0), (strip3a, 64, 0, 64, 0),
                        (strip3b, 0, 0, 63, 1)):
                    nc.sync.dma_start(
                        out=mk(st, p0, 1, 64, [[1, ns * 66]], d0 * 66),
                        in_=mk(h2buf, 0, 1, 64, [[1, ns * 66]], s0 * 66))

            def conv3_all(img):
                for ch in range(8):
                    ps = convps.tile([128, 512], F32, name="c3ps", tag="conv")
                    n_mm = 6 * NSPLIT
                    i = 0
                    for dx in range(3):
                        for w_, st3, pc in (("w3a", strip3a, 128),
                                            ("w3b", strip3b, 64)):
                            for sp in range(NSPLIT):
                                nc.tensor.matmul(
                                    ps, cs[w_][:, dx, sp, :],
                                    mk(st3, 0, 1, pc, [[66, 8], [1, 64]],
                                       8 * ch * 66 + dx),
                                    start=(i == 0), stop=(i == n_mm - 1))
                                i += 1
                    scr = wpool.tile([128, 512], F32, name="scr3", tag="scr3",
                                     bufs=2)
                    nc.scalar.activation(scr, ps, ActFn.Relu,
                                         bias=cs["cb3"][:, 0:1],
                                         accum_out=qacc[:, ch:ch + 1])
                nc.vector.reduce_sum(qT[:, img:img + 1], qacc, axis=AxX)

            # ---------------- software-pipelined image loop ----------------
            rep = int(os.environ.get("K_REP", "1"))
            loop_cm = tc.For_i(0, rep, 1) if rep > 1 else None
            if loop_cm is not None:
                loop_cm.__enter__()

            x_tiles = {0: load_x(0)}
            if b_loc > 1:
                x_tiles[1] = load_x(1)

            def load_misc_consts():
                for name, (shape, kind) in _const_specs().items():
                    if name not in crit:
                        nc.gpsimd.dma_start(out=cs[name],
                                            in_=const_d[name].ap())
            if os.environ.get("K_NOSKEW"):
                for i in range(b_loc):
                    yts = fft_step1(x_tiles[i])
                    fft_step2_mag(yts)
                    xm_to_dram_and_strips()
                    if i == 0:
                        load_misc_consts()
                    if i + 2 < b_loc:
                        x_tiles[i + 2] = load_x(i + 2)
                    conv1_half(0)
                    conv1_half(1)
                    fills2()
                    conv2_half(0)
                    conv2_half(1)
                    fills3()
                    conv3_all(i)
                    x_tiles.pop(i, None)
            else:
                # 3-deep skew: iteration i runs FFT(i+1), conv1(i),
                # conv2(i-1) and conv3(i-2), so every staging chain has at
                # least a full conv phase of matmuls to hide behind.
                yts = fft_step1(x_tiles[0])
                fft_step2_mag(yts)
                xm_to_dram_and_strips()
                load_misc_consts()
                for i in range(b_loc):
                    if i + 1 < b_loc:
                        yts = fft_step1(x_tiles[i + 1])
                    conv1_half(0)
                    conv1_half(1)
                    if i + 1 < b_loc:
                        fft_step2_mag(yts)
                        xm_to_dram_and_strips()
                    if i + 2 < b_loc:
                        x_tiles[i + 2] = load_x(i + 2)
                    if i >= 1:
                        conv2_half(0)
                        conv2_half(1)
                    fills2()
                    if i >= 2:
                        conv3_all(i - 2)
                    if i >= 1:
                        fills3()
                    x_tiles.pop(i, None)
                conv2_half(0)
                conv2_half(1)
                if b_loc >= 2:
                    conv3_all(b_loc - 2)
                fills3()
                conv3_all(b_loc - 1)

            # ---------------- retrieval (fp32) ----------------
            # key normalization -> knT [128, 400] (fp32)
            knT = rpool.tile([128, 400], F32, name="knT")
            ksq = rpool.tile([128, 4, 128], F32, name="ksq")
            nc.vector.tensor_mul(ksq, cs["keys"], cs["keys"])
            kss = rpool.tile([128, 4], F32, name="kss")
            nc.vector.reduce_sum(kss, ksq, axis=AxX)
            knm = rpool.tile([128, 4], F32, name="knm")
            nc.scalar.sqrt(knm, kss)
            nc.vector.tensor_scalar_max(knm, knm, 1e-12)
            kri = rpool.tile([128, 4], F32, name="kri")
            nc.vector.reciprocal(kri, knm)
            knrm = rpool.tile([128, 4, 128], F32, name="knrm")
            for c in range(4):
                nc.vector.tensor_scalar_mul(
                    knrm[:, c, :], cs["keys"][:, c, :], kri[:, c:c + 1])
            for c in range(4):
                pc = 128 if c < 3 else 16
                tp = miscps.tile([128, 128], F32, name="tp_kn", tag="misc")
                nc.tensor.transpose(
                    tp[:, :pc], knrm[:pc, c, :], cs["ident"][:pc, :pc])
                nc.scalar.copy(knT[:, c * 128:c * 128 + pc], tp[:, :pc])

            if qdbg_d is not None:
                nc.sync.dma_start(out=qdbg_d.ap(), in_=qT)
            if hdbg_d is not None:
                # dump last-image staging tiles raw (bf16)
                for nm_, src_ in (("dxm", xm), ("dh1", h1X), ("dh2", h2buf),
                                  ("ds2", strip2)):
                    nc.sync.dma_start(out=hdbg_d[nm_].ap(), in_=src_)
            bl = b_loc
            simps = miscps.tile([bl, 400], F32, name="simps", tag="misc")
            nc.tensor.matmul(simps, qT, knT, start=True, stop=True)
            gram = miscps.tile([bl, bl], F32, name="gram", tag="misc")
            nc.tensor.matmul(gram, qT, qT, start=True, stop=True)
            gd = rpool.tile([bl, bl], F32, name="gd")
            nc.vector.tensor_mul(gd, gram, cs["ident"][:bl, :bl])
            q2 = rpool.tile([bl, 1], F32, name="q2")
            nc.vector.reduce_sum(q2, gd, axis=AxX)
            qn = rpool.tile([bl, 1], F32, name="qn")
            nc.scalar.sqrt(qn, q2)
            nc.vector.tensor_scalar_max(qn, qn, 1e-12)
            rq = rpool.tile([bl, 1], F32, name="rq")
            nc.vector.reciprocal(rq, qn)
            sim = rpool.tile([bl, 400], F32, name="sim")
            nc.vector.tensor_scalar_mul(sim, simps, rq[:, 0:1])

            cur = rpool.tile([bl, 400], F32, name="cur")
            nc.vector.tensor_copy(cur, sim)
            m1 = rpool.tile([bl, 1], F32, name="m1")
            nc.vector.reduce_max(m1, sim, axis=AxX)
            msk = rpool.tile([bl, 400], F32, name="msk")
            mk_ = m1
            for it in range(4):
                nc.vector.tensor_scalar(msk, cur, mk_[:, 0:1], None,
                                        op0=AluOp.is_ge)
                nc.vector.scalar_tensor_tensor(cur, msk, -1e30, cur,
                                               op0=AluOp.mult, op1=AluOp.add)
                nm_ = rpool.tile([bl, 1], F32, name=f"mk{it}")
                nc.vector.reduce_max(nm_, cur, axis=AxX)
                mk_ = nm_
            m5 = mk_
            nc.vector.tensor_scalar(msk, sim, m5[:, 0:1], None, op0=AluOp.is_ge)
            m1n = rpool.tile([bl, 1], F32, name="m1n")
            nc.vector.tensor_scalar_mul(m1n, m1, -1.0)
            es = rpool.tile([bl, 400], F32, name="es")
            nc.scalar.activation(es, sim, ActFn.Exp, bias=m1n[:, 0:1])
            ew = rpool.tile([bl, 400], F32, name="ew")
            nc.vector.tensor_mul(ew, es, msk)
            zs = rpool.tile([bl, 1], F32, name="zs")
            nc.vector.reduce_sum(zs, ew, axis=AxX)
            rz = rpool.tile([bl, 1], F32, name="rz")
            nc.vector.reciprocal(rz, zs)
            nc.vector.tensor_scalar_mul(ew, ew, rz[:, 0:1])

            eT = rpool.tile([128, 4, bl], F32, name="eT")
            for c in range(4):
                pc = 128 if c < 3 else 16
                tp = miscps.tile([128, bl], F32, name="tp_e", tag="misc")
                nc.tensor.transpose(tp[:pc, :], ew[:, c * 128:c * 128 + pc],
                                    cs["ident"][:bl, :bl])
                nc.scalar.copy(eT[:pc, c, :], tp[:pc, :])

            memps = miscps.tile([128, bl], F32, name="memps", tag="misc")
            for c in range(4):
                pc = 128 if c < 3 else 16
                nc.tensor.matmul(memps, cs["vals"][:pc, c, :], eT[:pc, c, :],
                                 start=(c == 0), stop=(c == 3))
            memT = rpool.tile([128, bl], F32, name="memT")
            nc.scalar.copy(memT, memps)

            h1T = rpool.tile([128, 2, bl], F32, name="h1T")
            for mt in range(2):
                ps = miscps.tile([128, bl], F32, name="d1ps", tag="misc")
                nc.tensor.matmul(ps, cs["w1d"][:, mt * 128:(mt + 1) * 128],
                                 memT, start=True, stop=True)
                nc.scalar.activation(h1T[:, mt, :], ps, ActFn.Relu,
                                     bias=cs["b1d"][:, mt:mt + 1])
            h2T = rpool.tile([128, 4, bl], F32, name="h2T")
            for mt in range(4):
                ps = miscps.tile([128, bl], F32, name="d2ps", tag="misc")
                for kt in range(2):
                    nc.tensor.matmul(ps, cs["w2d"][:, kt, mt, :], h1T[:, kt, :],
                                     start=(kt == 0), stop=(kt == 1))
                nc.scalar.activation(h2T[:, mt, :], ps, ActFn.Relu,
                                     bias=cs["b2d"][:, mt:mt + 1])
            ops = miscps.tile([bl, 16], F32, name="outps", tag="misc")
            for c in range(4):
                nc.tensor.matmul(ops, h2T[:, c, :], cs["w3d"][:, c, :],
                                 start=(c == 0), stop=False)
            nc.tensor.matmul(ops, cs["ones1"][:, :bl], cs["b3row"],
                             start=False, stop=True)
            out_sb = rpool.tile([bl, 16], F32, name="out_sb")
            nc.scalar.copy(out_sb, ops)
            nc.sync.dma_start(out=out_d.ap(), in_=out_sb)
            if loop_cm is not None:
                loop_cm.__exit__(None, None, None)

    nc.compile()
    return nc


# ---------------------------------------------------------------------------
# host entry
# ---------------------------------------------------------------------------
_NC_CACHE = {}


def _get_nc(b_loc):
    key = (b_loc, STAGE, os.environ.get("K_REP", "1"),
           os.environ.get("K_NOSKEW"), os.environ.get("K_SYNCQ"),
           os.environ.get("K_DBGQ"), os.environ.get("K_DBGH"))
    if key not in _NC_CACHE:
        _NC_CACHE[key] = build_nc(b_loc)
    return _NC_CACHE[key]


def _pack_x(x_shard):
    b = x_shard.shape[0]
    xr = np.ascontiguousarray(
        x_shard.reshape(b, 2, 128, 256).transpose(0, 2, 1, 3)).astype(np.float32)
    return xr.astype(_np_sdt())


def kernel(**inputs):
    x = np.asarray(inputs["x"], np.float32)
    # jnp.fft.fftshift also shifts the batch axis: output b uses x[(b+64)%128]
    xp = np.roll(x, -64, axis=0)
    consts = _host_consts(inputs)

    b_loc = B // N_CORES
    nc = _get_nc(b_loc)

    in_maps = []
    for c in range(N_CORES):
        m = dict(consts)
        m["x_in"] = _pack_x(xp[c * b_loc:(c + 1) * b_loc])
        in_maps.append(m)

    kwargs = {}
    if os.environ.get("K_TRACE"):
        kwargs["trace"] = True
    res = run_bass_kernel_spmd(nc, in_maps, core_ids=list(range(N_CORES)),
                               **kwargs)
    global LAST_RESULTS
    LAST_RESULTS = res
    out = np.concatenate([r["out"] for r in res.results], axis=0)
    return out.reshape(B, 1, 4, 4).astype(np.float32)


LAST_RESULTS = None


if __name__ == "__main__":
    build_nc(int(os.environ.get("K_BLOC", "1")))
    print("built ok")
